# revision 22
# baseline (speedup 1.0000x reference)
"""Trainium2 Bass kernel for nn_CiderFeatures (all-pairs Gaussian reduction).

y[i, c] = norms[c] * sum_j exp(-(a_j + b[i,c]) * ||x_i - x_j||^2) * f_j

Key structure (from the reference constants A=D=2):
  a_j = beta_j  and  b_i = (beta_i/2, beta_i, 2*beta_i)  with
  beta = pi*(rho/2)^(2/3) * (2 + C2 * x),  so each channel weight is
  W_c[i,j] = exp(lnf_j - (beta_j + k_c beta_i) d2),  k_c in {1/2, 1, 2}.

Algorithm (identical program on all 8 cores, per-core data):
  - Host: balanced KD-tree sort -> 128-row i-blocks with tight boxes.
    Per (block, channel, j) culling with the EXACT worst-row bound
    f_j * exp(-min_i (beta_j + k_c beta_i) d2_ij), dropping the smallest
    until the dropped mass reaches EPS_DROP per row -- the Gaussians die
    within ~2 units while the cloud has radius ~9, so only ~4.4% of
    (pair, channel) terms survive.
  - Alive j's are gathered into dense chunks: 1024-wide "big" tiles,
    512-wide "small" tiles, and 256-wide "tiny" units merged PAIRWISE
    into one 512-wide tile (one exp instruction, two DVE reduces), so the
    per-instruction fixed cost (~410 ns on the bottleneck ScalarE) is
    amortized.  Tiles are balanced across cores and padded to equal
    counts, keeping the instruction stream identical on all cores.
  - Device, per tile: bf16 matmuls (K=28 rows: 10 logical dims x 2-level
    bf16 splits, per-tile centered coords, channel scale folded into the
    V side as exact powers of two) produce the exp argument [128, W] in
    PSUM; ScalarE computes exp in place.  Big tiles use the ScalarE
    accumulator for the j-sum (2-stage chain, 2 PSUM bufs suffice);
    small/pair tiles hand the sum to the otherwise idle VectorE (3-stage
    chain, 4-buf ring) which also avoids the 187 ns accumulator read.
  - Host scatters the per-tile [128,1] partials to rows, applies norms,
    undoes the sort.
"""

import numpy as np
import ml_dtypes
from math import pi

N_CORES = 8
IB = 128              # i-block rows (partition dim)
W_BIG = 1024          # big tile width (2 PSUM banks)
W_SMALL = 512         # small/pair tile width (1 PSUM bank)
W_TINY = 256          # tiny unit width (two per pair tile)
MM_N = 512            # matmul max output width (one PSUM bank)
K = 28                # contraction rows (10 dims, 2-level bf16 splits)
EPS_DROP = 5e-3       # max dropped |mass| per row per channel (absolute)
LNF_DEAD = -100.0
KCS = (0.5, 1.0, 2.0)   # channel scales k_c

_NC_CACHE = {}


# ---------------------------------------------------------------------------
# Host-side math
# ---------------------------------------------------------------------------

def _derived(rho, gamma, weights):
    B2 = 2.0
    C2 = (6.0 * pi ** 2) ** (2.0 / 3.0) * (6.0 * 2.0 / (160.0 * pi))
    rho_ = rho + 1e-8
    x = (gamma / (8.0 * rho_)) / (0.3 * (3.0 * pi ** 2) ** (2.0 / 3.0)
                                  * rho_ ** (5.0 / 3.0))
    scale = pi * (rho_ / 2.0) ** (2.0 / 3.0)
    beta = scale * (B2 + C2 * x)
    f = weights * rho
    lnf = np.maximum(np.log(np.maximum(f, 1e-300)), LNF_DEAD)
    Bs = np.array([2.0, 1.0, 2.0, 4.0])
    norms = ((Bs[0] + Bs[1:]) / 2.0) ** 1.5
    return beta, f, lnf, norms


def _kd_order(c, leaf=IB):
    """Balanced KD-tree order: leaves of `leaf` points with tight boxes."""
    out = []

    def rec(ids):
        if len(ids) <= leaf:
            out.append(ids)
            return
        ext = c[ids].max(0) - c[ids].min(0)
        srt = ids[np.argsort(c[ids, int(np.argmax(ext))], kind="stable")]
        half = (len(ids) // 2) // leaf * leaf
        if half == 0:
            half = leaf
        rec(srt[:half])
        rec(srt[half:])

    rec(np.arange(len(c)))
    return np.concatenate(out)


def _lev2(M):
    h0 = np.asarray(M, ml_dtypes.bfloat16).astype(np.float64)
    h1 = np.asarray(M - h0, ml_dtypes.bfloat16).astype(np.float64)
    return h0, h1


def _expand_rows(dims):
    """Rows: (v0,u0) always, (v0,u1) if u inexact, (v1,u0) if v inexact."""
    Vr, Ur = [], []
    for v, u, v_exact, u_exact in dims:
        v0, v1 = (v, None) if v_exact else _lev2(v)
        u0, u1 = (u, None) if u_exact else _lev2(u)
        Vr.append(v0); Ur.append(u0)
        if u1 is not None:
            Vr.append(v0); Ur.append(u1)
        if v1 is not None:
            Vr.append(v1); Ur.append(u0)
    return np.stack(Vr), np.stack(Ur)


def _tile_vu(xi, ri, kbi, xj, rj, bj, lj):
    """arg = lnf_j - (beta_j + k beta_i) d2, per-tile-centered coords.
    kbi = k_c * beta_i.  Row 0 pairs V=1 with the lnf dim (dead-col hook)."""
    one_i = np.ones_like(ri)
    one_j = np.ones_like(rj)
    dims = [
        (one_i, lj - bj * rj, True, False),        # rows 0,1
        (ri, -bj, False, False),
        (2.0 * xi[:, 0], bj * xj[:, 0], False, False),
        (2.0 * xi[:, 1], bj * xj[:, 1], False, False),
        (2.0 * xi[:, 2], bj * xj[:, 2], False, False),
        (-kbi * ri, one_j, False, True),
        (-kbi, rj, False, False),
        (2.0 * kbi * xi[:, 0], xj[:, 0], False, False),
        (2.0 * kbi * xi[:, 1], xj[:, 1], False, False),
        (2.0 * kbi * xi[:, 2], xj[:, 2], False, False),
    ]
    return _expand_rows(dims)


def _plan_widths(nb):
    """Counts per (1024-big, 512-small, 256-tiny) covering nb alive j's."""
    n1024, rem = divmod(nb, W_BIG)
    if rem == 0:
        return (n1024, 0, 0)
    if rem <= W_TINY:
        return (n1024, 0, 1)
    if rem <= W_SMALL:
        return (n1024, 1, 0)
    if rem <= W_SMALL + W_TINY:
        return (n1024, 1, 1)
    return (n1024 + 1, 0, 0)


def _interleave(counts):
    """Deterministic slot pattern (kind indices), each kind evenly spread."""
    total = sum(counts)
    emitted = [0] * len(counts)
    pat = []
    for t in range(total):
        best, bi = -1.0, 0
        for kk, nk in enumerate(counts):
            want = (t + 1) * nk / total - emitted[kk]
            if want > best:
                best, bi = want, kk
        pat.append(bi)
        emitted[bi] += 1
    return pat


def _prep_inputs(rho, gamma, coords, weights):
    rho = np.asarray(rho, np.float64)
    gamma = np.asarray(gamma, np.float64)
    coords = np.asarray(coords, np.float64)
    weights = np.asarray(weights, np.float64)
    n = rho.shape[0]
    beta, f, lnf, norms = _derived(rho, gamma, weights)

    order = _kd_order(coords)
    cs, bs_, lnfs, fs = coords[order], beta[order], lnf[order], f[order]
    nib = n // IB
    ib_lo = cs.reshape(nib, IB, 3).min(1)
    ib_hi = cs.reshape(nib, IB, 3).max(1)
    ib_c = 0.5 * (ib_lo + ib_hi)

    # exact worst-row culling bound, per (block, channel, j)
    cs32 = np.ascontiguousarray(cs, np.float32)
    bs32 = bs_.astype(np.float32)
    fs32 = fs.astype(np.float32)
    r32 = (cs32 ** 2).sum(1)
    units = []   # (block, channel, sorted alive j indices)
    for b in range(nib):
        ii = slice(b * IB, (b + 1) * IB)
        d2 = np.maximum(r32[ii][:, None] + r32[None, :]
                        - 2.0 * (cs32[ii] @ cs32.T), 0.0)
        for c, kc in enumerate(KCS):
            E = (bs32[None, :] + np.float32(kc) * bs32[ii][:, None]) * d2
            bound = fs32 * np.exp(-np.minimum(E.min(0), 80.0))
            srt = np.argsort(bound)
            cum = np.cumsum(bound[srt].astype(np.float64))
            nd = int(np.searchsorted(cum, EPS_DROP))
            idx = np.sort(srt[nd:])
            if len(idx):
                units.append((b, c, idx))

    # chunk units into bigs / smalls / tinies
    bigs, smalls, tinies = [], [], []
    for b, c, idx in units:
        nbig, nsmall, ntiny = _plan_widths(len(idx))
        pos = 0
        for lst, W_, cnt in ((bigs, W_BIG, nbig), (smalls, W_SMALL, nsmall),
                             (tinies, W_TINY, ntiny)):
            for _ in range(cnt):
                cj = np.full(W_, -1, np.int64)
                take = idx[pos:pos + W_]
                cj[:len(take)] = take
                lst.append((b, c, cj))
                pos += W_
    # merge tinies pairwise into 512-wide pair tiles
    DEAD_T = (-1, -1, np.full(W_TINY, -1, np.int64))
    if len(tinies) % 2:
        tinies.append(DEAD_T)
    pairs = [(tinies[2 * i], tinies[2 * i + 1])
             for i in range(len(tinies) // 2)]

    core_big = [[] for _ in range(N_CORES)]
    core_small = [[] for _ in range(N_CORES)]
    core_pair = [[] for _ in range(N_CORES)]
    for i, t in enumerate(bigs):
        core_big[i % N_CORES].append(t)
    for i, t in enumerate(smalls):
        core_small[i % N_CORES].append(t)
    for i, t in enumerate(pairs):
        core_pair[i % N_CORES].append(t)
    counts = (max(len(x) for x in core_big),
              max(len(x) for x in core_small),
              max(len(x) for x in core_pair))
    NBg, NSm, NPr = counts
    total_slots = NBg + NSm + 2 * NPr
    pattern = _interleave(counts)

    def build_tile(b, c, cj, W_):
        if b < 0:
            V = np.zeros((K, IB)); V[0:2] = 1.0
            U = np.zeros((K, W_)); U[0] = LNF_DEAD
            return V, U
        c_t = ib_c[b]
        ii = slice(b * IB, (b + 1) * IB)
        xi = cs[ii] - c_t
        ri = (xi ** 2).sum(1)
        kbi = KCS[c] * bs_[ii]
        real = cj >= 0
        jr = cj[real]
        xj = np.zeros((W_, 3)); rj = np.zeros(W_)
        bj = np.zeros(W_); lj = np.full(W_, LNF_DEAD)
        xj[real] = cs[jr] - c_t
        rj[real] = (xj[real] ** 2).sum(1)
        bj[real] = bs_[jr]
        lj[real] = lnfs[jr]
        return _tile_vu(xi, ri, kbi, xj, rj, bj, lj)

    in_maps = []
    tile_map = []   # per core: per slot, list of (block, channel) subunits
    for m in range(N_CORES):
        u0 = np.zeros((K, max(NBg, 1) * W_BIG), np.float64)
        v0 = np.zeros((K, max(NBg, 1) * IB), np.float64)
        u1 = np.zeros((K, max(NSm, 1) * W_SMALL), np.float64)
        v1 = np.zeros((K, max(NSm, 1) * IB), np.float64)
        u2 = np.zeros((K, max(NPr, 1) * W_SMALL), np.float64)
        v2 = np.zeros((K, max(NPr, 1) * 2 * IB), np.float64)
        tmap = []
        tk = [0, 0, 0]
        for kk in pattern:
            t = tk[kk]; tk[kk] += 1
            if kk == 0:
                lst = core_big[m]
                b, c, cj = lst[t] if t < len(lst) else \
                    (-1, -1, np.full(W_BIG, -1, np.int64))
                Vt, Ut = build_tile(b, c, cj, W_BIG)
                v0[:, t * IB:(t + 1) * IB] = Vt
                u0[:, t * W_BIG:(t + 1) * W_BIG] = Ut
                tmap.append([(b, c)])
            elif kk == 1:
                lst = core_small[m]
                b, c, cj = lst[t] if t < len(lst) else \
                    (-1, -1, np.full(W_SMALL, -1, np.int64))
                Vt, Ut = build_tile(b, c, cj, W_SMALL)
                v1[:, t * IB:(t + 1) * IB] = Vt
                u1[:, t * W_SMALL:(t + 1) * W_SMALL] = Ut
                tmap.append([(b, c)])
            else:
                lst = core_pair[m]
                t1, t2 = lst[t] if t < len(lst) else (DEAD_T, DEAD_T)
                subs = []
                for h, (b, c, cj) in enumerate((t1, t2)):
                    Vt, Ut = build_tile(b, c, cj, W_TINY)
                    v2[:, (2 * t + h) * IB:(2 * t + h + 1) * IB] = Vt
                    u2[:, t * W_SMALL + h * W_TINY:
                       t * W_SMALL + (h + 1) * W_TINY] = Ut
                    subs.append((b, c))
                tmap.append(subs)
        tile_map.append(tmap)
        bft = ml_dtypes.bfloat16
        in_maps.append({
            "u0": np.ascontiguousarray(u0.astype(bft)),
            "v0": np.ascontiguousarray(v0.astype(bft)),
            "u1": np.ascontiguousarray(u1.astype(bft)),
            "v1": np.ascontiguousarray(v1.astype(bft)),
            "u2": np.ascontiguousarray(u2.astype(bft)),
            "v2": np.ascontiguousarray(v2.astype(bft)),
        })
    meta = dict(order=order, tile_map=tile_map, norms=norms, n=n,
                counts=counts, total_slots=total_slots)
    return meta, in_maps


# ---------------------------------------------------------------------------
# Device kernel
# ---------------------------------------------------------------------------

def _build_nc(counts, repeat=1):
    import concourse.bass as bass  # noqa: F401
    import concourse.tile as tile
    from concourse import bacc, mybir

    NBg, NSm, NPr = counts
    total_slots = NBg + NSm + 2 * NPr
    nc = bacc.Bacc("TRN2", target_bir_lowering=False)
    u0_d = nc.dram_tensor("u0", [K, max(NBg, 1) * W_BIG], mybir.dt.bfloat16,
                          kind="ExternalInput")
    v0_d = nc.dram_tensor("v0", [K, max(NBg, 1) * IB], mybir.dt.bfloat16,
                          kind="ExternalInput")
    u1_d = nc.dram_tensor("u1", [K, max(NSm, 1) * W_SMALL], mybir.dt.bfloat16,
                          kind="ExternalInput")
    v1_d = nc.dram_tensor("v1", [K, max(NSm, 1) * IB], mybir.dt.bfloat16,
                          kind="ExternalInput")
    u2_d = nc.dram_tensor("u2", [K, max(NPr, 1) * W_SMALL], mybir.dt.bfloat16,
                          kind="ExternalInput")
    v2_d = nc.dram_tensor("v2", [K, max(NPr, 1) * 2 * IB], mybir.dt.bfloat16,
                          kind="ExternalInput")
    y_dram = nc.dram_tensor("y", [IB, total_slots], mybir.dt.float32,
                            kind="ExternalOutput")

    NDMA = 8
    pattern = _interleave(counts)

    with tile.TileContext(nc) as tc:
        with (
            tc.tile_pool(name="singles", bufs=1) as singles,
            tc.tile_pool(name="ps0", bufs=2, space="PSUM") as ps0,   # 4 banks
            tc.tile_pool(name="ps1", bufs=4, space="PSUM") as ps1,   # 4 banks
        ):
            warm = singles.tile([128, 1], mybir.dt.float32)
            nc.vector.memset(warm[:], 0.0)
            nc.scalar.activation(out=warm[:], in_=warm[:],
                                 func=mybir.ActivationFunctionType.Exp)

            def load_v(dram, cols, tag):
                t = singles.tile([K, cols], mybir.dt.bfloat16, tag=tag)
                nc.sync.dma_start(t[:], dram[:, 0:cols])
                return t

            v_sb = (load_v(v0_d, max(NBg, 1) * IB, "v0"),
                    load_v(v1_d, max(NSm, 1) * IB, "v1"),
                    load_v(v2_d, max(NPr, 1) * 2 * IB, "v2"))

            def stage_u(dram, T, W_, name):
                ct = max(1, (T + NDMA - 1) // NDMA)
                outs = []
                for cch in range(NDMA):
                    lo = cch * ct * W_
                    hi = min(T, (cch + 1) * ct) * W_
                    if lo >= hi:
                        break
                    ut = singles.tile([K, hi - lo], mybir.dt.bfloat16,
                                      tag=f"{name}_{cch}")
                    nc.sync.dma_start(ut[:], dram[:, lo:hi])
                    outs.append(ut)
                return outs, ct

            u_sb = (stage_u(u0_d, NBg, W_BIG, "u0"),
                    stage_u(u1_d, NSm, W_SMALL, "u1"),
                    stage_u(u2_d, NPr, W_SMALL, "u2"))
            parts = singles.tile([IB, total_slots], mybir.dt.float32)

            Exp = mybir.ActivationFunctionType.Exp
            for _ in range(repeat):
                tk = [0, 0, 0]
                for kk in pattern:
                    t = tk[kk]; tk[kk] += 1
                    u_tiles, ct = u_sb[kk]
                    if kk == 0:
                        uc = u_tiles[t // ct][:, (t % ct) * W_BIG:
                                              (t % ct + 1) * W_BIG]
                        pt = ps0.tile([IB, W_BIG], mybir.dt.float32, tag="p0")
                        for q in range(W_BIG // MM_N):
                            nc.tensor.matmul(
                                pt[:, q * MM_N:(q + 1) * MM_N],
                                v_sb[0][:, t * IB:(t + 1) * IB],
                                uc[:, q * MM_N:(q + 1) * MM_N],
                                start=True, stop=True)
                        nc.scalar.activation(out=pt[:], in_=pt[:], func=Exp,
                                             accum_out=parts[:, t:t + 1])
                    elif kk == 1:
                        uc = u_tiles[t // ct][:, (t % ct) * W_SMALL:
                                              (t % ct + 1) * W_SMALL]
                        pt = ps1.tile([IB, W_SMALL], mybir.dt.float32,
                                      tag="p1")
                        nc.tensor.matmul(pt[:],
                                         v_sb[1][:, t * IB:(t + 1) * IB],
                                         uc, start=True, stop=True)
                        nc.scalar.activation(out=pt[:], in_=pt[:], func=Exp)
                        slot = NBg + t
                        nc.vector.reduce_sum(parts[:, slot:slot + 1], pt[:],
                                             axis=mybir.AxisListType.X)
                    else:
                        uc = u_tiles[t // ct][:, (t % ct) * W_SMALL:
                                              (t % ct + 1) * W_SMALL]
                        pt = ps1.tile([IB, W_SMALL], mybir.dt.float32,
                                      tag="p1")
                        for h in range(2):
                            nc.tensor.matmul(
                                pt[:, h * W_TINY:(h + 1) * W_TINY],
                                v_sb[2][:, (2 * t + h) * IB:
                                        (2 * t + h + 1) * IB],
                                uc[:, h * W_TINY:(h + 1) * W_TINY],
                                start=True, stop=True)
                        nc.scalar.activation(out=pt[:], in_=pt[:], func=Exp)
                        for h in range(2):
                            slot = NBg + NSm + 2 * t + h
                            nc.vector.reduce_sum(
                                parts[:, slot:slot + 1],
                                pt[:, h * W_TINY:(h + 1) * W_TINY],
                                axis=mybir.AxisListType.X)
            nc.sync.dma_start(y_dram[:], parts[:])
    nc.finalize()
    return nc


def _assemble(meta, results):
    n = meta["n"]
    order, tile_map, norms = meta["order"], meta["tile_map"], meta["norms"]
    NBg, NSm, NPr = meta["counts"]
    pattern = _interleave(meta["counts"])
    Ys = np.zeros((n, 3), np.float64)
    for m, res in enumerate(results):
        y_dev = np.asarray(res["y"], np.float64)
        tk = [0, 0, 0]
        for slot_i, kk in enumerate(pattern):
            t = tk[kk]; tk[kk] += 1
            subs = tile_map[m][slot_i]
            if kk == 0:
                cols = [t]
            elif kk == 1:
                cols = [NBg + t]
            else:
                cols = [NBg + NSm + 2 * t, NBg + NSm + 2 * t + 1]
            for (b, c), col in zip(subs, cols):
                if b < 0:
                    continue
                Ys[b * IB:(b + 1) * IB, c] += y_dev[:, col]
    Ys *= norms[None, :]
    out = np.empty((n, 3), np.float32)
    out[order] = Ys.astype(np.float32)
    return out


def kernel_run(rho, gamma, coords, weights, **spmd_kwargs):
    """Run on hardware; returns (y, BassKernelResults)."""
    from concourse.bass_utils import run_bass_kernel_spmd

    meta, in_maps = _prep_inputs(rho, gamma, coords, weights)
    key = meta["counts"]
    if key not in _NC_CACHE:
        _NC_CACHE[key] = _build_nc(key)
    res = run_bass_kernel_spmd(_NC_CACHE[key], in_maps,
                               core_ids=list(range(N_CORES)), **spmd_kwargs)
    return _assemble(meta, res.results), res


def kernel(rho, gamma, coords, weights):
    y, _ = kernel_run(rho, gamma, coords, weights)
    return y


# revision 24
# speedup vs baseline: 1.1213x; 1.1213x over previous
"""Trainium2 Bass kernel for nn_CiderFeatures (all-pairs Gaussian reduction).

y[i, c] = norms[c] * sum_j exp(-(a_j + b[i,c]) * ||x_i - x_j||^2) * f_j

Key structure (from the reference constants A=D=2):
  a_j = beta_j  and  b_i = (beta_i/2, beta_i, 2*beta_i)  with
  beta = pi*(rho/2)^(2/3) * (2 + C2 * x),  so each channel weight is
  W_c[i,j] = exp(lnf_j - (beta_j + k_c beta_i) d2),  k_c in {1/2, 1, 2}.

Algorithm (identical program on all 8 cores, per-core data):
  - Host: balanced KD-tree sort -> 128-row i-blocks with tight boxes.
    Per (block, channel, j) culling with the EXACT worst-row bound
    f_j * exp(-min_i (beta_j + k_c beta_i) d2_ij), dropping the smallest
    until the dropped mass reaches EPS_DROP per row -- the Gaussians die
    within ~2 units while the cloud has radius ~9, so only ~4.4% of
    (pair, channel) terms survive.
  - Alive j's are gathered into dense chunks: 1024-wide "big" tiles,
    512-wide "small" tiles, and 256-wide "tiny" units merged PAIRWISE
    into one 512-wide tile (one exp instruction, two DVE reduces), so the
    per-instruction fixed cost (~410 ns on the bottleneck ScalarE) is
    amortized.  Tiles are balanced across cores and padded to equal
    counts, keeping the instruction stream identical on all cores.
  - Device, per tile: bf16 matmuls (K=28 rows: 10 logical dims x 2-level
    bf16 splits, per-tile centered coords, channel scale folded into the
    V side as exact powers of two) produce the exp argument [128, W] in
    PSUM; ScalarE computes exp in place.  Big tiles use the ScalarE
    accumulator for the j-sum (2-stage chain, 2 PSUM bufs suffice);
    small/pair tiles hand the sum to the otherwise idle VectorE (3-stage
    chain, 4-buf ring) which also avoids the 187 ns accumulator read.
  - Host scatters the per-tile [128,1] partials to rows, applies norms,
    undoes the sort.
"""

import numpy as np
import ml_dtypes
from math import pi

N_CORES = 8
IB = 128              # i-block rows (partition dim)
W_BIG = 1024          # big tile width (2 PSUM banks)
W_SMALL = 512         # small/pair tile width (1 PSUM bank)
W_TINY = 256          # tiny unit width (two per pair tile)
MM_N = 512            # matmul max output width (one PSUM bank)
K = 28                # contraction rows (10 dims, 2-level bf16 splits)
EPS_DROP = 5e-3       # max dropped |mass| per row per channel (absolute)
LNF_DEAD = -100.0
KCS = (0.5, 1.0, 2.0)   # channel scales k_c

_NC_CACHE = {}


# ---------------------------------------------------------------------------
# Host-side math
# ---------------------------------------------------------------------------

def _derived(rho, gamma, weights):
    B2 = 2.0
    C2 = (6.0 * pi ** 2) ** (2.0 / 3.0) * (6.0 * 2.0 / (160.0 * pi))
    rho_ = rho + 1e-8
    x = (gamma / (8.0 * rho_)) / (0.3 * (3.0 * pi ** 2) ** (2.0 / 3.0)
                                  * rho_ ** (5.0 / 3.0))
    scale = pi * (rho_ / 2.0) ** (2.0 / 3.0)
    beta = scale * (B2 + C2 * x)
    f = weights * rho
    lnf = np.maximum(np.log(np.maximum(f, 1e-300)), LNF_DEAD)
    Bs = np.array([2.0, 1.0, 2.0, 4.0])
    norms = ((Bs[0] + Bs[1:]) / 2.0) ** 1.5
    return beta, f, lnf, norms


def _kd_order(c, leaf=IB):
    """Balanced KD-tree order: leaves of `leaf` points with tight boxes."""
    out = []

    def rec(ids):
        if len(ids) <= leaf:
            out.append(ids)
            return
        ext = c[ids].max(0) - c[ids].min(0)
        srt = ids[np.argsort(c[ids, int(np.argmax(ext))], kind="stable")]
        half = (len(ids) // 2) // leaf * leaf
        if half == 0:
            half = leaf
        rec(srt[:half])
        rec(srt[half:])

    rec(np.arange(len(c)))
    return np.concatenate(out)


def _lev2(M):
    h0 = np.asarray(M, ml_dtypes.bfloat16).astype(np.float64)
    h1 = np.asarray(M - h0, ml_dtypes.bfloat16).astype(np.float64)
    return h0, h1


def _expand_rows(dims):
    """Rows: (v0,u0) always, (v0,u1) if u inexact, (v1,u0) if v inexact."""
    Vr, Ur = [], []
    for v, u, v_exact, u_exact in dims:
        v0, v1 = (v, None) if v_exact else _lev2(v)
        u0, u1 = (u, None) if u_exact else _lev2(u)
        Vr.append(v0); Ur.append(u0)
        if u1 is not None:
            Vr.append(v0); Ur.append(u1)
        if v1 is not None:
            Vr.append(v1); Ur.append(u0)
    return np.stack(Vr), np.stack(Ur)


def _tile_vu(xi, ri, kbi, xj, rj, bj, lj):
    """arg = lnf_j - (beta_j + k beta_i) d2, per-tile-centered coords.
    kbi = k_c * beta_i.  Row 0 pairs V=1 with the lnf dim (dead-col hook)."""
    one_i = np.ones_like(ri)
    one_j = np.ones_like(rj)
    dims = [
        (one_i, lj - bj * rj, True, False),        # rows 0,1
        (ri, -bj, False, False),
        (2.0 * xi[:, 0], bj * xj[:, 0], False, False),
        (2.0 * xi[:, 1], bj * xj[:, 1], False, False),
        (2.0 * xi[:, 2], bj * xj[:, 2], False, False),
        (-kbi * ri, one_j, False, True),
        (-kbi, rj, False, False),
        (2.0 * kbi * xi[:, 0], xj[:, 0], False, False),
        (2.0 * kbi * xi[:, 1], xj[:, 1], False, False),
        (2.0 * kbi * xi[:, 2], xj[:, 2], False, False),
    ]
    return _expand_rows(dims)


def _plan_widths(nb):
    """Counts per (1024-big, 512-small, 256-tiny) covering nb alive j's."""
    n1024, rem = divmod(nb, W_BIG)
    if rem == 0:
        return (n1024, 0, 0)
    if rem <= W_TINY:
        return (n1024, 0, 1)
    if rem <= W_SMALL:
        return (n1024, 1, 0)
    if rem <= W_SMALL + W_TINY:
        return (n1024, 1, 1)
    return (n1024 + 1, 0, 0)


def _interleave(counts):
    """Deterministic slot pattern (kind indices), each kind evenly spread."""
    total = sum(counts)
    emitted = [0] * len(counts)
    pat = []
    for t in range(total):
        best, bi = -1.0, 0
        for kk, nk in enumerate(counts):
            want = (t + 1) * nk / total - emitted[kk]
            if want > best:
                best, bi = want, kk
        pat.append(bi)
        emitted[bi] += 1
    return pat


def _prep_inputs(rho, gamma, coords, weights):
    rho = np.asarray(rho, np.float64)
    gamma = np.asarray(gamma, np.float64)
    coords = np.asarray(coords, np.float64)
    weights = np.asarray(weights, np.float64)
    n = rho.shape[0]
    beta, f, lnf, norms = _derived(rho, gamma, weights)

    order = _kd_order(coords)
    cs, bs_, lnfs, fs = coords[order], beta[order], lnf[order], f[order]
    nib = n // IB
    ib_lo = cs.reshape(nib, IB, 3).min(1)
    ib_hi = cs.reshape(nib, IB, 3).max(1)
    ib_c = 0.5 * (ib_lo + ib_hi)

    # exact worst-row culling bound, per (block, channel, j)
    cs32 = np.ascontiguousarray(cs, np.float32)
    bs32 = bs_.astype(np.float32)
    fs32 = fs.astype(np.float32)
    r32 = (cs32 ** 2).sum(1)
    units = []   # (block, channel, sorted alive j indices)
    for b in range(nib):
        ii = slice(b * IB, (b + 1) * IB)
        d2 = np.maximum(r32[ii][:, None] + r32[None, :]
                        - 2.0 * (cs32[ii] @ cs32.T), 0.0)
        for c, kc in enumerate(KCS):
            E = (bs32[None, :] + np.float32(kc) * bs32[ii][:, None]) * d2
            bound = fs32 * np.exp(-np.minimum(E.min(0), 80.0))
            srt = np.argsort(bound)
            cum = np.cumsum(bound[srt].astype(np.float64))
            nd = int(np.searchsorted(cum, EPS_DROP))
            idx = np.sort(srt[nd:])
            if len(idx):
                units.append((b, c, idx))

    # chunk units into bigs / smalls / tinies
    bigs, smalls, tinies = [], [], []
    for b, c, idx in units:
        nbig, nsmall, ntiny = _plan_widths(len(idx))
        pos = 0
        for lst, W_, cnt in ((bigs, W_BIG, nbig), (smalls, W_SMALL, nsmall),
                             (tinies, W_TINY, ntiny)):
            for _ in range(cnt):
                cj = np.full(W_, -1, np.int64)
                take = idx[pos:pos + W_]
                cj[:len(take)] = take
                lst.append((b, c, cj))
                pos += W_
    # merge tinies pairwise into 512-wide pair tiles
    DEAD_T = (-1, -1, np.full(W_TINY, -1, np.int64))
    if len(tinies) % 2:
        tinies.append(DEAD_T)
    pairs = [(tinies[2 * i], tinies[2 * i + 1])
             for i in range(len(tinies) // 2)]

    core_big = [[] for _ in range(N_CORES)]
    core_small = [[] for _ in range(N_CORES)]
    core_pair = [[] for _ in range(N_CORES)]
    for i, t in enumerate(bigs):
        core_big[i % N_CORES].append(t)
    for i, t in enumerate(smalls):
        core_small[i % N_CORES].append(t)
    for i, t in enumerate(pairs):
        core_pair[i % N_CORES].append(t)
    counts = (max(len(x) for x in core_big),
              max(len(x) for x in core_small),
              max(len(x) for x in core_pair))
    NBg, NSm, NPr = counts
    total_slots = NBg + NSm + 2 * NPr
    pattern = _interleave(counts)

    def build_tile(b, c, cj, W_):
        if b < 0:
            V = np.zeros((K, IB)); V[0:2] = 1.0
            U = np.zeros((K, W_)); U[0] = LNF_DEAD
            return V, U
        c_t = ib_c[b]
        ii = slice(b * IB, (b + 1) * IB)
        xi = cs[ii] - c_t
        ri = (xi ** 2).sum(1)
        kbi = KCS[c] * bs_[ii]
        real = cj >= 0
        jr = cj[real]
        xj = np.zeros((W_, 3)); rj = np.zeros(W_)
        bj = np.zeros(W_); lj = np.full(W_, LNF_DEAD)
        xj[real] = cs[jr] - c_t
        rj[real] = (xj[real] ** 2).sum(1)
        bj[real] = bs_[jr]
        lj[real] = lnfs[jr]
        return _tile_vu(xi, ri, kbi, xj, rj, bj, lj)

    in_maps = []
    tile_map = []   # per core: per slot, list of (block, channel) subunits
    for m in range(N_CORES):
        u0 = np.zeros((K, max(NBg, 1) * W_BIG), np.float64)
        v0 = np.zeros((K, max(NBg, 1) * IB), np.float64)
        u1 = np.zeros((K, max(NSm, 1) * W_SMALL), np.float64)
        v1 = np.zeros((K, max(NSm, 1) * IB), np.float64)
        u2 = np.zeros((K, max(NPr, 1) * W_SMALL), np.float64)
        v2 = np.zeros((K, max(NPr, 1) * 2 * IB), np.float64)
        tmap = []
        tk = [0, 0, 0]
        for kk in pattern:
            t = tk[kk]; tk[kk] += 1
            if kk == 0:
                lst = core_big[m]
                b, c, cj = lst[t] if t < len(lst) else \
                    (-1, -1, np.full(W_BIG, -1, np.int64))
                Vt, Ut = build_tile(b, c, cj, W_BIG)
                v0[:, t * IB:(t + 1) * IB] = Vt
                u0[:, t * W_BIG:(t + 1) * W_BIG] = Ut
                tmap.append([(b, c)])
            elif kk == 1:
                lst = core_small[m]
                b, c, cj = lst[t] if t < len(lst) else \
                    (-1, -1, np.full(W_SMALL, -1, np.int64))
                Vt, Ut = build_tile(b, c, cj, W_SMALL)
                v1[:, t * IB:(t + 1) * IB] = Vt
                u1[:, t * W_SMALL:(t + 1) * W_SMALL] = Ut
                tmap.append([(b, c)])
            else:
                lst = core_pair[m]
                t1, t2 = lst[t] if t < len(lst) else (DEAD_T, DEAD_T)
                subs = []
                for h, (b, c, cj) in enumerate((t1, t2)):
                    Vt, Ut = build_tile(b, c, cj, W_TINY)
                    v2[:, (2 * t + h) * IB:(2 * t + h + 1) * IB] = Vt
                    u2[:, t * W_SMALL + h * W_TINY:
                       t * W_SMALL + (h + 1) * W_TINY] = Ut
                    subs.append((b, c))
                tmap.append(subs)
        tile_map.append(tmap)
        bft = ml_dtypes.bfloat16
        in_maps.append({
            "u0": np.ascontiguousarray(u0.astype(bft)),
            "v0": np.ascontiguousarray(v0.astype(bft)),
            "u1": np.ascontiguousarray(u1.astype(bft)),
            "v1": np.ascontiguousarray(v1.astype(bft)),
            "u2": np.ascontiguousarray(u2.astype(bft)),
            "v2": np.ascontiguousarray(v2.astype(bft)),
        })
    meta = dict(order=order, tile_map=tile_map, norms=norms, n=n,
                counts=counts, total_slots=total_slots)
    return meta, in_maps


# ---------------------------------------------------------------------------
# Device kernel
# ---------------------------------------------------------------------------

def _build_nc(counts, repeat=1):
    import concourse.bass as bass  # noqa: F401
    import concourse.tile as tile
    from concourse import bacc, mybir

    NBg, NSm, NPr = counts
    total_slots = NBg + NSm + 2 * NPr
    nc = bacc.Bacc("TRN2", target_bir_lowering=False)
    u0_d = nc.dram_tensor("u0", [K, max(NBg, 1) * W_BIG], mybir.dt.bfloat16,
                          kind="ExternalInput")
    v0_d = nc.dram_tensor("v0", [K, max(NBg, 1) * IB], mybir.dt.bfloat16,
                          kind="ExternalInput")
    u1_d = nc.dram_tensor("u1", [K, max(NSm, 1) * W_SMALL], mybir.dt.bfloat16,
                          kind="ExternalInput")
    v1_d = nc.dram_tensor("v1", [K, max(NSm, 1) * IB], mybir.dt.bfloat16,
                          kind="ExternalInput")
    u2_d = nc.dram_tensor("u2", [K, max(NPr, 1) * W_SMALL], mybir.dt.bfloat16,
                          kind="ExternalInput")
    v2_d = nc.dram_tensor("v2", [K, max(NPr, 1) * 2 * IB], mybir.dt.bfloat16,
                          kind="ExternalInput")
    y_dram = nc.dram_tensor("y", [IB, total_slots], mybir.dt.float32,
                            kind="ExternalOutput")

    NDMA = 8
    pattern = _interleave(counts)
    mult = mybir.AluOpType.mult
    add_op = mybir.AluOpType.add

    with tile.TileContext(nc) as tc:
        with (
            tc.tile_pool(name="singles", bufs=1) as singles,
            tc.tile_pool(name="ps0", bufs=2, space="PSUM") as ps0,   # 4 banks
            tc.tile_pool(name="ps1", bufs=4, space="PSUM") as ps1,   # 4 banks
            tc.tile_pool(name="sgb", bufs=3) as sgb_pool,
            tc.tile_pool(name="sgs", bufs=4) as sgs_pool,
        ):
            warm = singles.tile([128, 1], mybir.dt.float32)
            nc.vector.memset(warm[:], 0.0)
            nc.scalar.activation(out=warm[:], in_=warm[:],
                                 func=mybir.ActivationFunctionType.Exp)

            def load_v(dram, cols, tag):
                t = singles.tile([K, cols], mybir.dt.bfloat16, tag=tag)
                nc.sync.dma_start(t[:], dram[:, 0:cols])
                return t

            v_sb = (load_v(v0_d, max(NBg, 1) * IB, "v0"),
                    load_v(v1_d, max(NSm, 1) * IB, "v1"),
                    load_v(v2_d, max(NPr, 1) * 2 * IB, "v2"))

            def stage_u(dram, T, W_, name):
                ct = max(1, (T + NDMA - 1) // NDMA)
                outs = []
                for cch in range(NDMA):
                    lo = cch * ct * W_
                    hi = min(T, (cch + 1) * ct) * W_
                    if lo >= hi:
                        break
                    ut = singles.tile([K, hi - lo], mybir.dt.bfloat16,
                                      tag=f"{name}_{cch}")
                    nc.sync.dma_start(ut[:], dram[:, lo:hi])
                    outs.append(ut)
                return outs, ct

            u_sb = (stage_u(u0_d, NBg, W_BIG, "u0"),
                    stage_u(u1_d, NSm, W_SMALL, "u1"),
                    stage_u(u2_d, NPr, W_SMALL, "u2"))
            parts = singles.tile([IB, total_slots], mybir.dt.float32)

            Exp = mybir.ActivationFunctionType.Exp
            for _ in range(repeat):
                tk = [0, 0, 0]
                for kk in pattern:
                    t = tk[kk]; tk[kk] += 1
                    u_tiles, ct = u_sb[kk]
                    if kk == 0:
                        uc = u_tiles[t // ct][:, (t % ct) * W_BIG:
                                              (t % ct + 1) * W_BIG]
                        pt = ps0.tile([IB, W_BIG], mybir.dt.float32, tag="p0")
                        for q in range(W_BIG // MM_N):
                            nc.tensor.matmul(
                                pt[:, q * MM_N:(q + 1) * MM_N],
                                v_sb[0][:, t * IB:(t + 1) * IB],
                                uc[:, q * MM_N:(q + 1) * MM_N],
                                start=True, stop=True)
                        sg = sgb_pool.tile([IB, W_BIG], mybir.dt.bfloat16,
                                           tag="sgb")
                        nc.scalar.activation(out=sg[:], in_=pt[:], func=Exp)
                        nc.vector.tensor_scalar(
                            out=sg[:], in0=sg[:], scalar1=1.0, scalar2=0.0,
                            op0=mult, op1=add_op,
                            accum_out=parts[:, t:t + 1])
                    elif kk == 1:
                        uc = u_tiles[t // ct][:, (t % ct) * W_SMALL:
                                              (t % ct + 1) * W_SMALL]
                        pt = ps1.tile([IB, W_SMALL], mybir.dt.float32,
                                      tag="p1")
                        nc.tensor.matmul(pt[:],
                                         v_sb[1][:, t * IB:(t + 1) * IB],
                                         uc, start=True, stop=True)
                        sg = sgs_pool.tile([IB, W_SMALL], mybir.dt.bfloat16,
                                           tag="sgs")
                        nc.scalar.activation(out=sg[:], in_=pt[:], func=Exp)
                        slot = NBg + t
                        nc.vector.tensor_scalar(
                            out=sg[:], in0=sg[:], scalar1=1.0, scalar2=0.0,
                            op0=mult, op1=add_op,
                            accum_out=parts[:, slot:slot + 1])
                    else:
                        uc = u_tiles[t // ct][:, (t % ct) * W_SMALL:
                                              (t % ct + 1) * W_SMALL]
                        pt = ps1.tile([IB, W_SMALL], mybir.dt.float32,
                                      tag="p1")
                        for h in range(2):
                            nc.tensor.matmul(
                                pt[:, h * W_TINY:(h + 1) * W_TINY],
                                v_sb[2][:, (2 * t + h) * IB:
                                        (2 * t + h + 1) * IB],
                                uc[:, h * W_TINY:(h + 1) * W_TINY],
                                start=True, stop=True)
                        sg = sgs_pool.tile([IB, W_SMALL], mybir.dt.bfloat16,
                                           tag="sgs")
                        nc.scalar.activation(out=sg[:], in_=pt[:], func=Exp)
                        for h in range(2):
                            slot = NBg + NSm + 2 * t + h
                            hv = sg[:, h * W_TINY:(h + 1) * W_TINY]
                            nc.vector.tensor_scalar(
                                out=hv, in0=hv, scalar1=1.0, scalar2=0.0,
                                op0=mult, op1=add_op,
                                accum_out=parts[:, slot:slot + 1])
            nc.sync.dma_start(y_dram[:], parts[:])
    nc.finalize()
    return nc


def _assemble(meta, results):
    n = meta["n"]
    order, tile_map, norms = meta["order"], meta["tile_map"], meta["norms"]
    NBg, NSm, NPr = meta["counts"]
    pattern = _interleave(meta["counts"])
    Ys = np.zeros((n, 3), np.float64)
    for m, res in enumerate(results):
        y_dev = np.asarray(res["y"], np.float64)
        tk = [0, 0, 0]
        for slot_i, kk in enumerate(pattern):
            t = tk[kk]; tk[kk] += 1
            subs = tile_map[m][slot_i]
            if kk == 0:
                cols = [t]
            elif kk == 1:
                cols = [NBg + t]
            else:
                cols = [NBg + NSm + 2 * t, NBg + NSm + 2 * t + 1]
            for (b, c), col in zip(subs, cols):
                if b < 0:
                    continue
                Ys[b * IB:(b + 1) * IB, c] += y_dev[:, col]
    Ys *= norms[None, :]
    out = np.empty((n, 3), np.float32)
    out[order] = Ys.astype(np.float32)
    return out


def kernel_run(rho, gamma, coords, weights, **spmd_kwargs):
    """Run on hardware; returns (y, BassKernelResults)."""
    from concourse.bass_utils import run_bass_kernel_spmd

    meta, in_maps = _prep_inputs(rho, gamma, coords, weights)
    key = meta["counts"]
    if key not in _NC_CACHE:
        _NC_CACHE[key] = _build_nc(key)
    res = run_bass_kernel_spmd(_NC_CACHE[key], in_maps,
                               core_ids=list(range(N_CORES)), **spmd_kwargs)
    return _assemble(meta, res.results), res


def kernel(rho, gamma, coords, weights):
    y, _ = kernel_run(rho, gamma, coords, weights)
    return y


# revision 25
# speedup vs baseline: 1.2286x; 1.0956x over previous
"""Trainium2 Bass kernel for nn_CiderFeatures (all-pairs Gaussian reduction).

y[i, c] = norms[c] * sum_j exp(-(a_j + b[i,c]) * ||x_i - x_j||^2) * f_j

Key structure (from the reference constants A=D=2):
  a_j = beta_j  and  b_i = (beta_i/2, beta_i, 2*beta_i)  with
  beta = pi*(rho/2)^(2/3) * (2 + C2 * x),  so each channel weight is
  W_c[i,j] = exp(lnf_j - (beta_j + k_c beta_i) d2),  k_c in {1/2, 1, 2}.

Algorithm (identical program on all 8 cores, per-core data):
  - Host: balanced KD-tree sort -> 128-row i-blocks with tight boxes.
    Per (block, channel, j) culling with the EXACT worst-row bound
    f_j * exp(-min_i (beta_j + k_c beta_i) d2_ij), dropping the smallest
    until the dropped mass reaches EPS_DROP per row -- the Gaussians die
    within ~2 units while the cloud has radius ~9, so only ~4.4% of
    (pair, channel) terms survive.
  - Each (block, channel) unit's alive j's are packed into 256-wide
    SLOTS (last slot padded with dead columns); slots are distributed
    across cores unit-wise (LPT) and concatenated into a uniform stream
    of 2048-wide tiles (8 slots each).  All cores run the identical
    instruction stream on per-core packed data; slot counts are padded.
  - Device, per tile: 8 bf16 matmuls (one per slot, K=28 contraction
    rows: 10 logical dims x 2-level bf16 splits, per-tile-centered
    coords, channel scale folded into the V side as exact powers of two)
    fill a [128, 2048] PSUM tile with exp arguments; ONE ScalarE exp
    covers the whole tile (fixed costs amortized 8x); VectorE runs one
    4x-mode tensor_scalar per slot whose free accumulator emits the
    [128,1] j-sum.  ScalarE is the bottleneck at ~95% occupancy.
  - Host scatters the per-slot partials to rows, applies norms, undoes
    the sort.
"""

import numpy as np
import ml_dtypes
from math import pi

N_CORES = 8
IB = 128              # i-block rows (partition dim)
W_SLOT = 256          # unit slot width
SPT = 8               # slots per 2048-wide tile
W_TILE = W_SLOT * SPT
K = 28                # contraction rows (10 dims, 2-level bf16 splits)
EPS_DROP = 5e-3       # max dropped |mass| per row per channel (absolute)
LNF_DEAD = -100.0
KCS = (0.5, 1.0, 2.0)   # channel scales k_c

_NC_CACHE = {}


# ---------------------------------------------------------------------------
# Host-side math
# ---------------------------------------------------------------------------

def _derived(rho, gamma, weights):
    B2 = 2.0
    C2 = (6.0 * pi ** 2) ** (2.0 / 3.0) * (6.0 * 2.0 / (160.0 * pi))
    rho_ = rho + 1e-8
    x = (gamma / (8.0 * rho_)) / (0.3 * (3.0 * pi ** 2) ** (2.0 / 3.0)
                                  * rho_ ** (5.0 / 3.0))
    scale = pi * (rho_ / 2.0) ** (2.0 / 3.0)
    beta = scale * (B2 + C2 * x)
    f = weights * rho
    lnf = np.maximum(np.log(np.maximum(f, 1e-300)), LNF_DEAD)
    Bs = np.array([2.0, 1.0, 2.0, 4.0])
    norms = ((Bs[0] + Bs[1:]) / 2.0) ** 1.5
    return beta, f, lnf, norms


def _kd_order(c, leaf=IB):
    """Balanced KD-tree order: leaves of `leaf` points with tight boxes."""
    out = []

    def rec(ids):
        if len(ids) <= leaf:
            out.append(ids)
            return
        ext = c[ids].max(0) - c[ids].min(0)
        srt = ids[np.argsort(c[ids, int(np.argmax(ext))], kind="stable")]
        half = (len(ids) // 2) // leaf * leaf
        if half == 0:
            half = leaf
        rec(srt[:half])
        rec(srt[half:])

    rec(np.arange(len(c)))
    return np.concatenate(out)


def _lev2(M):
    h0 = np.asarray(M, ml_dtypes.bfloat16).astype(np.float64)
    h1 = np.asarray(M - h0, ml_dtypes.bfloat16).astype(np.float64)
    return h0, h1


def _expand_rows(dims):
    """Rows: (v0,u0) always, (v0,u1) if u inexact, (v1,u0) if v inexact."""
    Vr, Ur = [], []
    for v, u, v_exact, u_exact in dims:
        v0, v1 = (v, None) if v_exact else _lev2(v)
        u0, u1 = (u, None) if u_exact else _lev2(u)
        Vr.append(v0); Ur.append(u0)
        if u1 is not None:
            Vr.append(v0); Ur.append(u1)
        if v1 is not None:
            Vr.append(v1); Ur.append(u0)
    return np.stack(Vr), np.stack(Ur)


def _tile_vu(xi, ri, kbi, xj, rj, bj, lj):
    """arg = lnf_j - (beta_j + k beta_i) d2, per-tile-centered coords.
    kbi = k_c * beta_i.  Row 0 pairs V=1 with the lnf dim (dead-col hook)."""
    one_i = np.ones_like(ri)
    one_j = np.ones_like(rj)
    dims = [
        (one_i, lj - bj * rj, True, False),        # rows 0,1
        (ri, -bj, False, False),
        (2.0 * xi[:, 0], bj * xj[:, 0], False, False),
        (2.0 * xi[:, 1], bj * xj[:, 1], False, False),
        (2.0 * xi[:, 2], bj * xj[:, 2], False, False),
        (-kbi * ri, one_j, False, True),
        (-kbi, rj, False, False),
        (2.0 * kbi * xi[:, 0], xj[:, 0], False, False),
        (2.0 * kbi * xi[:, 1], xj[:, 1], False, False),
        (2.0 * kbi * xi[:, 2], xj[:, 2], False, False),
    ]
    return _expand_rows(dims)


def _prep_inputs(rho, gamma, coords, weights):
    rho = np.asarray(rho, np.float64)
    gamma = np.asarray(gamma, np.float64)
    coords = np.asarray(coords, np.float64)
    weights = np.asarray(weights, np.float64)
    n = rho.shape[0]
    beta, f, lnf, norms = _derived(rho, gamma, weights)

    order = _kd_order(coords)
    cs, bs_, lnfs, fs = coords[order], beta[order], lnf[order], f[order]
    nib = n // IB
    ib_lo = cs.reshape(nib, IB, 3).min(1)
    ib_hi = cs.reshape(nib, IB, 3).max(1)
    ib_c = 0.5 * (ib_lo + ib_hi)

    # exact worst-row culling bound, per (block, channel, j)
    cs32 = np.ascontiguousarray(cs, np.float32)
    bs32 = bs_.astype(np.float32)
    fs32 = fs.astype(np.float32)
    r32 = (cs32 ** 2).sum(1)
    units = []   # (block, channel, sorted alive j indices)
    for b in range(nib):
        ii = slice(b * IB, (b + 1) * IB)
        d2 = np.maximum(r32[ii][:, None] + r32[None, :]
                        - 2.0 * (cs32[ii] @ cs32.T), 0.0)
        for c, kc in enumerate(KCS):
            E = (bs32[None, :] + np.float32(kc) * bs32[ii][:, None]) * d2
            bound = fs32 * np.exp(-np.minimum(E.min(0), 80.0))
            srt = np.argsort(bound)
            cum = np.cumsum(bound[srt].astype(np.float64))
            nd = int(np.searchsorted(cum, EPS_DROP))
            idx = np.sort(srt[nd:])
            if len(idx):
                units.append((b, c, idx))

    # units -> 256-wide slots, distributed unit-wise across cores (LPT)
    unit_slots = [(b, c, idx, (len(idx) + W_SLOT - 1) // W_SLOT)
                  for b, c, idx in units]
    unit_slots.sort(key=lambda u: -u[3])
    core_units = [[] for _ in range(N_CORES)]
    loads = np.zeros(N_CORES, np.int64)
    for b, c, idx, ns in unit_slots:
        m = int(np.argmin(loads))
        loads[m] += ns
        core_units[m].append((b, c, idx, ns))
    S = int(-(-loads.max() // SPT) * SPT)     # slots per core, 8-aligned
    T = S // SPT

    in_maps = []
    slot_map = []   # per core: (block, channel) per slot
    for m in range(N_CORES):
        U = np.zeros((K, S * W_SLOT), np.float64)
        V = np.zeros((K, S * IB), np.float64)
        smap = []
        s = 0
        for b, c, idx, ns in core_units[m]:
            c_t = ib_c[b]
            ii = slice(b * IB, (b + 1) * IB)
            xi = cs[ii] - c_t
            ri = (xi ** 2).sum(1)
            kbi = KCS[c] * bs_[ii]
            W_ = ns * W_SLOT
            real = np.zeros(W_, bool)
            real[:len(idx)] = True
            xj = np.zeros((W_, 3)); rj = np.zeros(W_)
            bj = np.zeros(W_); lj = np.full(W_, LNF_DEAD)
            xj[real] = cs[idx] - c_t
            rj[real] = (xj[real] ** 2).sum(1)
            bj[real] = bs_[idx]
            lj[real] = lnfs[idx]
            Vt, Ut = _tile_vu(xi, ri, kbi, xj, rj, bj, lj)
            U[:, s * W_SLOT:s * W_SLOT + W_] = Ut
            V[:, s * IB:(s + ns) * IB] = np.tile(Vt, (1, ns))
            smap += [(b, c)] * ns
            s += ns
        for sd in range(s, S):                 # dead padding slots
            V[0:2, sd * IB:(sd + 1) * IB] = 1.0
            U[0, sd * W_SLOT:(sd + 1) * W_SLOT] = LNF_DEAD
            smap.append((-1, -1))
        slot_map.append(smap)
        bft = ml_dtypes.bfloat16
        in_maps.append({
            "u": np.ascontiguousarray(U.astype(bft)),
            "v": np.ascontiguousarray(V.astype(bft)),
        })
    meta = dict(order=order, slot_map=slot_map, norms=norms, n=n, S=S, T=T)
    return meta, in_maps


# ---------------------------------------------------------------------------
# Device kernel
# ---------------------------------------------------------------------------

def _build_nc(S, repeat=1):
    import concourse.bass as bass  # noqa: F401
    import concourse.tile as tile
    from concourse import bacc, mybir

    T = S // SPT
    nc = bacc.Bacc("TRN2", target_bir_lowering=False)
    u_dram = nc.dram_tensor("u", [K, S * W_SLOT], mybir.dt.bfloat16,
                            kind="ExternalInput")
    v_dram = nc.dram_tensor("v", [K, S * IB], mybir.dt.bfloat16,
                            kind="ExternalInput")
    y_dram = nc.dram_tensor("y", [IB, S], mybir.dt.float32,
                            kind="ExternalOutput")

    NDMA = 8
    mult = mybir.AluOpType.mult
    add_op = mybir.AluOpType.add

    with tile.TileContext(nc) as tc:
        with (
            tc.tile_pool(name="singles", bufs=1) as singles,
            tc.tile_pool(name="ps", bufs=2, space="PSUM") as ps_pool,
            tc.tile_pool(name="sg", bufs=3) as sg_pool,
        ):
            warm = singles.tile([128, 1], mybir.dt.float32)
            nc.vector.memset(warm[:], 0.0)
            nc.scalar.activation(out=warm[:], in_=warm[:],
                                 func=mybir.ActivationFunctionType.Exp)

            v_sb = singles.tile([K, S * IB], mybir.dt.bfloat16, tag="v")
            nc.sync.dma_start(v_sb[:], v_dram[:])
            ct = max(1, (T + NDMA - 1) // NDMA)   # tiles per u chunk
            u_tiles = []
            for cch in range(NDMA):
                lo = cch * ct * W_TILE
                hi = min(T, (cch + 1) * ct) * W_TILE
                if lo >= hi:
                    break
                ut = singles.tile([K, hi - lo], mybir.dt.bfloat16,
                                  tag=f"u{cch}")
                nc.sync.dma_start(ut[:], u_dram[:, lo:hi])
                u_tiles.append(ut)
            parts = singles.tile([IB, S], mybir.dt.float32)

            Exp = mybir.ActivationFunctionType.Exp
            for _ in range(repeat):
                for t in range(T):
                    uc = u_tiles[t // ct]
                    ubase = (t % ct) * W_TILE
                    pt = ps_pool.tile([IB, W_TILE], mybir.dt.float32,
                                      tag="ps")
                    for s8 in range(SPT):
                        s = t * SPT + s8
                        nc.tensor.matmul(
                            pt[:, s8 * W_SLOT:(s8 + 1) * W_SLOT],
                            v_sb[:, s * IB:(s + 1) * IB],
                            uc[:, ubase + s8 * W_SLOT:
                               ubase + (s8 + 1) * W_SLOT],
                            start=True, stop=True)
                    sg = sg_pool.tile([IB, W_TILE], mybir.dt.bfloat16,
                                      tag="sg")
                    nc.scalar.activation(out=sg[:], in_=pt[:], func=Exp)
                    for s8 in range(SPT):
                        s = t * SPT + s8
                        hv = sg[:, s8 * W_SLOT:(s8 + 1) * W_SLOT]
                        nc.vector.tensor_scalar(
                            out=hv, in0=hv, scalar1=1.0, scalar2=0.0,
                            op0=mult, op1=add_op,
                            accum_out=parts[:, s:s + 1])
            nc.sync.dma_start(y_dram[:], parts[:])
    nc.finalize()
    return nc


def _assemble(meta, results):
    n = meta["n"]
    order, slot_map, norms = meta["order"], meta["slot_map"], meta["norms"]
    Ys = np.zeros((n, 3), np.float64)
    for m, res in enumerate(results):
        y_dev = np.asarray(res["y"], np.float64)       # [128, S]
        for s, (b, c) in enumerate(slot_map[m]):
            if b < 0:
                continue
            Ys[b * IB:(b + 1) * IB, c] += y_dev[:, s]
    Ys *= norms[None, :]
    out = np.empty((n, 3), np.float32)
    out[order] = Ys.astype(np.float32)
    return out


def kernel_run(rho, gamma, coords, weights, **spmd_kwargs):
    """Run on hardware; returns (y, BassKernelResults)."""
    from concourse.bass_utils import run_bass_kernel_spmd

    meta, in_maps = _prep_inputs(rho, gamma, coords, weights)
    key = meta["S"]
    if key not in _NC_CACHE:
        _NC_CACHE[key] = _build_nc(key)
    res = run_bass_kernel_spmd(_NC_CACHE[key], in_maps,
                               core_ids=list(range(N_CORES)), **spmd_kwargs)
    return _assemble(meta, res.results), res


def kernel(rho, gamma, coords, weights):
    y, _ = kernel_run(rho, gamma, coords, weights)
    return y


# revision 27
# speedup vs baseline: 1.3143x; 1.0698x over previous
"""Trainium2 Bass kernel for nn_CiderFeatures (all-pairs Gaussian reduction).

y[i, c] = norms[c] * sum_j exp(-(a_j + b[i,c]) * ||x_i - x_j||^2) * f_j

Key structure (from the reference constants A=D=2):
  a_j = beta_j  and  b_i = (beta_i/2, beta_i, 2*beta_i)  with
  beta = pi*(rho/2)^(2/3) * (2 + C2 * x),  so each channel weight is
  W_c[i,j] = exp(lnf_j - (beta_j + k_c beta_i) d2),  k_c in {1/2, 1, 2}.

Algorithm (identical program on all 8 cores, per-core data):
  - Host: balanced KD-tree sort -> 128-row i-blocks with tight boxes.
    Per (block, channel, j) culling with the EXACT worst-row bound
    f_j * exp(-min_i (beta_j + k_c beta_i) d2_ij), dropping the smallest
    until the dropped mass reaches EPS_DROP per row -- the Gaussians die
    within ~2 units while the cloud has radius ~9, so only ~4.4% of
    (pair, channel) terms survive.
  - Each (block, channel) unit's alive j's are packed into 256-wide
    SLOTS (last slot padded with dead columns); slots are distributed
    across cores unit-wise (LPT) and concatenated into a uniform stream
    of 2048-wide tiles (8 slots each).  All cores run the identical
    instruction stream on per-core packed data; slot counts are padded.
  - Device, per tile: 8 bf16 matmuls (one per slot, K=28 contraction
    rows: 10 logical dims x 2-level bf16 splits, per-tile-centered
    coords, channel scale folded into the V side as exact powers of two)
    fill a [128, 2048] PSUM tile with exp arguments; ONE ScalarE exp
    covers the whole tile (fixed costs amortized 8x); VectorE runs one
    4x-mode tensor_scalar per slot whose free accumulator emits the
    [128,1] j-sum.  ScalarE is the bottleneck at ~95% occupancy.
  - Host scatters the per-slot partials to rows, applies norms, undoes
    the sort.
"""

import numpy as np
import ml_dtypes
from math import pi

N_CORES = 8
IB = 128              # i-block rows (partition dim)
W_SLOT = 256          # unit slot width
SPT = 8               # slots per 2048-wide tile
W_TILE = W_SLOT * SPT
K = 28                # contraction rows (10 dims, 2-level bf16 splits)
EPS_DROP = 5e-3       # max dropped |mass| per row per channel (absolute)
LNF_DEAD = -100.0
KCS = (0.5, 1.0, 2.0)   # channel scales k_c

_NC_CACHE = {}


# ---------------------------------------------------------------------------
# Host-side math
# ---------------------------------------------------------------------------

def _derived(rho, gamma, weights):
    B2 = 2.0
    C2 = (6.0 * pi ** 2) ** (2.0 / 3.0) * (6.0 * 2.0 / (160.0 * pi))
    rho_ = rho + 1e-8
    x = (gamma / (8.0 * rho_)) / (0.3 * (3.0 * pi ** 2) ** (2.0 / 3.0)
                                  * rho_ ** (5.0 / 3.0))
    scale = pi * (rho_ / 2.0) ** (2.0 / 3.0)
    beta = scale * (B2 + C2 * x)
    f = weights * rho
    lnf = np.maximum(np.log(np.maximum(f, 1e-300)), LNF_DEAD)
    Bs = np.array([2.0, 1.0, 2.0, 4.0])
    norms = ((Bs[0] + Bs[1:]) / 2.0) ** 1.5
    return beta, f, lnf, norms


def _kd_order(c, leaf=IB):
    """Balanced KD-tree order: leaves of `leaf` points with tight boxes."""
    out = []

    def rec(ids):
        if len(ids) <= leaf:
            out.append(ids)
            return
        ext = c[ids].max(0) - c[ids].min(0)
        srt = ids[np.argsort(c[ids, int(np.argmax(ext))], kind="stable")]
        half = (len(ids) // 2) // leaf * leaf
        if half == 0:
            half = leaf
        rec(srt[:half])
        rec(srt[half:])

    rec(np.arange(len(c)))
    return np.concatenate(out)


def _lev2(M):
    h0 = np.asarray(M, ml_dtypes.bfloat16).astype(np.float64)
    h1 = np.asarray(M - h0, ml_dtypes.bfloat16).astype(np.float64)
    return h0, h1


def _expand_rows(dims):
    """Rows: (v0,u0) always, (v0,u1) if u inexact, (v1,u0) if v inexact."""
    Vr, Ur = [], []
    for v, u, v_exact, u_exact in dims:
        v0, v1 = (v, None) if v_exact else _lev2(v)
        u0, u1 = (u, None) if u_exact else _lev2(u)
        Vr.append(v0); Ur.append(u0)
        if u1 is not None:
            Vr.append(v0); Ur.append(u1)
        if v1 is not None:
            Vr.append(v1); Ur.append(u0)
    return np.stack(Vr), np.stack(Ur)


def _tile_vu(xi, ri, kbi, xj, rj, bj, lj):
    """arg = lnf_j - (beta_j + k beta_i) d2, per-tile-centered coords.
    kbi = k_c * beta_i.  Row 0 pairs V=1 with the lnf dim (dead-col hook)."""
    one_i = np.ones_like(ri)
    one_j = np.ones_like(rj)
    dims = [
        (one_i, lj - bj * rj, True, False),        # rows 0,1
        (ri, -bj, False, False),
        (2.0 * xi[:, 0], bj * xj[:, 0], False, False),
        (2.0 * xi[:, 1], bj * xj[:, 1], False, False),
        (2.0 * xi[:, 2], bj * xj[:, 2], False, False),
        (-kbi * ri, one_j, False, True),
        (-kbi, rj, False, False),
        (2.0 * kbi * xi[:, 0], xj[:, 0], False, False),
        (2.0 * kbi * xi[:, 1], xj[:, 1], False, False),
        (2.0 * kbi * xi[:, 2], xj[:, 2], False, False),
    ]
    return _expand_rows(dims)


def _prep_inputs(rho, gamma, coords, weights):
    rho = np.asarray(rho, np.float64)
    gamma = np.asarray(gamma, np.float64)
    coords = np.asarray(coords, np.float64)
    weights = np.asarray(weights, np.float64)
    n = rho.shape[0]
    beta, f, lnf, norms = _derived(rho, gamma, weights)

    order = _kd_order(coords)
    cs, bs_, lnfs, fs = coords[order], beta[order], lnf[order], f[order]
    nib = n // IB
    ib_lo = cs.reshape(nib, IB, 3).min(1)
    ib_hi = cs.reshape(nib, IB, 3).max(1)
    ib_c = 0.5 * (ib_lo + ib_hi)

    # exact worst-row culling bound, per (block, channel, j)
    cs32 = np.ascontiguousarray(cs, np.float32)
    bs32 = bs_.astype(np.float32)
    fs32 = fs.astype(np.float32)
    r32 = (cs32 ** 2).sum(1)
    units = []   # (block, channel, sorted alive j indices)
    for b in range(nib):
        ii = slice(b * IB, (b + 1) * IB)
        d2 = np.maximum(r32[ii][:, None] + r32[None, :]
                        - 2.0 * (cs32[ii] @ cs32.T), 0.0)
        for c, kc in enumerate(KCS):
            E = (bs32[None, :] + np.float32(kc) * bs32[ii][:, None]) * d2
            bound = fs32 * np.exp(-np.minimum(E.min(0), 80.0))
            srt = np.argsort(bound)
            cum = np.cumsum(bound[srt].astype(np.float64))
            nd = int(np.searchsorted(cum, EPS_DROP))
            idx = np.sort(srt[nd:])
            if len(idx):
                units.append((b, c, idx))

    # units -> 256-wide slots, distributed unit-wise across cores (LPT)
    unit_slots = [(b, c, idx, (len(idx) + W_SLOT - 1) // W_SLOT)
                  for b, c, idx in units]
    unit_slots.sort(key=lambda u: -u[3])
    core_units = [[] for _ in range(N_CORES)]
    loads = np.zeros(N_CORES, np.int64)
    for b, c, idx, ns in unit_slots:
        m = int(np.argmin(loads))
        loads[m] += ns
        core_units[m].append((b, c, idx, ns))
    S = int(-(-loads.max() // SPT) * SPT)     # slots per core, 8-aligned
    T = S // SPT

    in_maps = []
    slot_map = []   # per core: (block, channel) per slot
    for m in range(N_CORES):
        U = np.zeros((K, S * W_SLOT), np.float64)
        V = np.zeros((K, S * IB), np.float64)
        smap = []
        s = 0
        for b, c, idx, ns in core_units[m]:
            c_t = ib_c[b]
            ii = slice(b * IB, (b + 1) * IB)
            xi = cs[ii] - c_t
            ri = (xi ** 2).sum(1)
            kbi = KCS[c] * bs_[ii]
            W_ = ns * W_SLOT
            real = np.zeros(W_, bool)
            real[:len(idx)] = True
            xj = np.zeros((W_, 3)); rj = np.zeros(W_)
            bj = np.zeros(W_); lj = np.full(W_, LNF_DEAD)
            xj[real] = cs[idx] - c_t
            rj[real] = (xj[real] ** 2).sum(1)
            bj[real] = bs_[idx]
            lj[real] = lnfs[idx]
            Vt, Ut = _tile_vu(xi, ri, kbi, xj, rj, bj, lj)
            U[:, s * W_SLOT:s * W_SLOT + W_] = Ut
            V[:, s * IB:(s + ns) * IB] = np.tile(Vt, (1, ns))
            smap += [(b, c)] * ns
            s += ns
        for sd in range(s, S):                 # dead padding slots
            V[0:2, sd * IB:(sd + 1) * IB] = 1.0
            U[0, sd * W_SLOT:(sd + 1) * W_SLOT] = LNF_DEAD
            smap.append((-1, -1))
        slot_map.append(smap)
        bft = ml_dtypes.bfloat16
        in_maps.append({
            "u": np.ascontiguousarray(U.astype(bft)),
            "v": np.ascontiguousarray(V.astype(bft)),
        })
    meta = dict(order=order, slot_map=slot_map, norms=norms, n=n, S=S, T=T)
    return meta, in_maps


# ---------------------------------------------------------------------------
# Device kernel
# ---------------------------------------------------------------------------

def _build_nc(S, repeat=1):
    import concourse.bass as bass  # noqa: F401
    import concourse.tile as tile
    from concourse import bacc, mybir

    T = S // SPT
    nc = bacc.Bacc("TRN2", target_bir_lowering=False)
    u_dram = nc.dram_tensor("u", [K, S * W_SLOT], mybir.dt.bfloat16,
                            kind="ExternalInput")
    v_dram = nc.dram_tensor("v", [K, S * IB], mybir.dt.bfloat16,
                            kind="ExternalInput")
    y_dram = nc.dram_tensor("y", [IB, S], mybir.dt.float32,
                            kind="ExternalOutput")

    NDMA = 8
    mult = mybir.AluOpType.mult
    add_op = mybir.AluOpType.add

    with tile.TileContext(nc) as tc:
        with (
            tc.tile_pool(name="singles", bufs=1) as singles,
            tc.tile_pool(name="ps", bufs=2, space="PSUM") as ps_pool,
            tc.tile_pool(name="sg", bufs=3) as sg_pool,
        ):
            warm = singles.tile([128, 1], mybir.dt.float32)
            nc.vector.memset(warm[:], 0.0)
            nc.scalar.activation(out=warm[:], in_=warm[:],
                                 func=mybir.ActivationFunctionType.Exp)

            ct = max(1, (T + NDMA - 1) // NDMA)   # tiles per chunk
            u_tiles, v_tiles = [], []
            for cch in range(NDMA):
                lo = cch * ct
                hi = min(T, (cch + 1) * ct)
                if lo >= hi:
                    break
                vt = singles.tile([K, (hi - lo) * SPT * IB],
                                  mybir.dt.bfloat16, tag=f"v{cch}")
                nc.sync.dma_start(vt[:], v_dram[:, lo * SPT * IB:
                                                hi * SPT * IB])
                v_tiles.append(vt)
                ut = singles.tile([K, (hi - lo) * W_TILE],
                                  mybir.dt.bfloat16, tag=f"u{cch}")
                nc.sync.dma_start(ut[:], u_dram[:, lo * W_TILE:hi * W_TILE])
                u_tiles.append(ut)
            parts = singles.tile([IB, S], mybir.dt.float32)

            Exp = mybir.ActivationFunctionType.Exp
            for _ in range(repeat):
                for t in range(T):
                    uc = u_tiles[t // ct]
                    vc = v_tiles[t // ct]
                    ubase = (t % ct) * W_TILE
                    vbase = (t % ct) * SPT * IB
                    pt = ps_pool.tile([IB, W_TILE], mybir.dt.float32,
                                      tag="ps")
                    for s8 in range(SPT):
                        nc.tensor.matmul(
                            pt[:, s8 * W_SLOT:(s8 + 1) * W_SLOT],
                            vc[:, vbase + s8 * IB:vbase + (s8 + 1) * IB],
                            uc[:, ubase + s8 * W_SLOT:
                               ubase + (s8 + 1) * W_SLOT],
                            start=True, stop=True)
                    sg = sg_pool.tile([IB, W_TILE], mybir.dt.bfloat16,
                                      tag="sg")
                    nc.scalar.activation(out=sg[:], in_=pt[:], func=Exp)
                    for s8 in range(SPT):
                        s = t * SPT + s8
                        hv = sg[:, s8 * W_SLOT:(s8 + 1) * W_SLOT]
                        nc.vector.tensor_scalar(
                            out=hv, in0=hv, scalar1=1.0, scalar2=0.0,
                            op0=mult, op1=add_op,
                            accum_out=parts[:, s:s + 1])
            nc.sync.dma_start(y_dram[:], parts[:])
    nc.finalize()
    return nc


def _assemble(meta, results):
    n = meta["n"]
    order, slot_map, norms = meta["order"], meta["slot_map"], meta["norms"]
    Ys = np.zeros((n, 3), np.float64)
    for m, res in enumerate(results):
        y_dev = np.asarray(res["y"], np.float64)       # [128, S]
        for s, (b, c) in enumerate(slot_map[m]):
            if b < 0:
                continue
            Ys[b * IB:(b + 1) * IB, c] += y_dev[:, s]
    Ys *= norms[None, :]
    out = np.empty((n, 3), np.float32)
    out[order] = Ys.astype(np.float32)
    return out


def kernel_run(rho, gamma, coords, weights, **spmd_kwargs):
    """Run on hardware; returns (y, BassKernelResults)."""
    from concourse.bass_utils import run_bass_kernel_spmd

    meta, in_maps = _prep_inputs(rho, gamma, coords, weights)
    key = meta["S"]
    if key not in _NC_CACHE:
        _NC_CACHE[key] = _build_nc(key)
    res = run_bass_kernel_spmd(_NC_CACHE[key], in_maps,
                               core_ids=list(range(N_CORES)), **spmd_kwargs)
    return _assemble(meta, res.results), res


def kernel(rho, gamma, coords, weights):
    y, _ = kernel_run(rho, gamma, coords, weights)
    return y


# revision 36
# speedup vs baseline: 1.4225x; 1.0823x over previous
"""Trainium2 Bass kernel for nn_CiderFeatures (all-pairs Gaussian reduction).

y[i, c] = norms[c] * sum_j exp(-(a_j + b[i,c]) * ||x_i - x_j||^2) * f_j

Key structure (from the reference constants A=D=2):
  a_j = beta_j  and  b_i = (beta_i/2, beta_i, 2*beta_i)  with
  beta = pi*(rho/2)^(2/3) * (2 + C2 * x),  so each channel weight is
  W_c[i,j] = exp(lnf_j - (beta_j + k_c beta_i) d2),  k_c in {1/2, 1, 2}.

Algorithm (identical program on all 8 cores, per-core data):
  - Host: balanced KD-tree sort -> 128-row i-blocks with tight boxes.
    Per (block, channel, j) culling with the EXACT worst-row bound
    f_j * exp(-min_i (beta_j + k_c beta_i) d2_ij), dropping the smallest
    until the dropped mass reaches EPS_DROP per row -- the Gaussians die
    within ~2 units while the cloud has radius ~9, so only ~4.4% of
    (pair, channel) terms survive.
  - Each (block, channel) unit's alive j's are packed into 256-wide
    SLOTS (last slot padded with dead columns); slots are distributed
    across cores unit-wise (LPT) and concatenated into a uniform stream
    of 2048-wide tiles (8 slots each).  All cores run the identical
    instruction stream on per-core packed data; slot counts are padded.
  - Device, per tile: 8 bf16 matmuls (one per slot, K=28 contraction
    rows: 10 logical dims x 2-level bf16 splits, per-tile-centered
    coords, channel scale folded into the V side as exact powers of two)
    fill a [128, 2048] PSUM tile with exp arguments; ONE ScalarE exp
    covers the whole tile (fixed costs amortized 8x); VectorE runs one
    4x-mode tensor_scalar per slot whose free accumulator emits the
    [128,1] j-sum.  ScalarE is the bottleneck at ~95% occupancy.
  - Host scatters the per-slot partials to rows, applies norms, undoes
    the sort.
"""

import numpy as np
import ml_dtypes
from math import pi

N_CORES = 8
IB = 128              # i-block rows (partition dim)
W_SLOT = 256          # unit slot width
SPT = 8               # slots per 2048-wide tile
W_TILE = W_SLOT * SPT
K = 28                # contraction rows (10 dims, 2-level bf16 splits)
EPS_DROP = 5e-3       # max dropped |mass| per row per channel (absolute)
LNF_DEAD = -100.0
KCS = (0.5, 1.0, 2.0)   # channel scales k_c

# int16-Schraudolph exp on VectorE for a fraction of tiles: bf16 bits of
# exp(arg) ~= int16(arg * 128/ln2 + B16), computed as a DVE convert with
# the bias+clamp fused; K16 is folded into the matmul host-side.
K16 = 128.0 / np.log(2.0)
B16 = 128.0 * (127.0 - 0.0583)   # sigma tuned for ~zero mass-weighted bias


def _dve_tiles(T):
    """Tile indices whose exp runs on VectorE (engine balance)."""
    Y = int(round(T / 4.0))
    return sorted({min(T - 1, int((i + 0.5) * T / Y)) for i in range(Y)}) \
        if Y else []


_NC_CACHE = {}


# ---------------------------------------------------------------------------
# Host-side math
# ---------------------------------------------------------------------------

def _derived(rho, gamma, weights):
    B2 = 2.0
    C2 = (6.0 * pi ** 2) ** (2.0 / 3.0) * (6.0 * 2.0 / (160.0 * pi))
    rho_ = rho + 1e-8
    x = (gamma / (8.0 * rho_)) / (0.3 * (3.0 * pi ** 2) ** (2.0 / 3.0)
                                  * rho_ ** (5.0 / 3.0))
    scale = pi * (rho_ / 2.0) ** (2.0 / 3.0)
    beta = scale * (B2 + C2 * x)
    f = weights * rho
    lnf = np.maximum(np.log(np.maximum(f, 1e-300)), LNF_DEAD)
    Bs = np.array([2.0, 1.0, 2.0, 4.0])
    norms = ((Bs[0] + Bs[1:]) / 2.0) ** 1.5
    return beta, f, lnf, norms


def _kd_order(c, leaf=IB):
    """Balanced KD-tree order: leaves of `leaf` points with tight boxes."""
    out = []

    def rec(ids):
        if len(ids) <= leaf:
            out.append(ids)
            return
        ext = c[ids].max(0) - c[ids].min(0)
        srt = ids[np.argsort(c[ids, int(np.argmax(ext))], kind="stable")]
        half = (len(ids) // 2) // leaf * leaf
        if half == 0:
            half = leaf
        rec(srt[:half])
        rec(srt[half:])

    rec(np.arange(len(c)))
    return np.concatenate(out)


def _lev2(M):
    h0 = np.asarray(M, ml_dtypes.bfloat16).astype(np.float64)
    h1 = np.asarray(M - h0, ml_dtypes.bfloat16).astype(np.float64)
    return h0, h1


def _expand_rows(dims):
    """Rows: (v0,u0) always, (v0,u1) if u inexact, (v1,u0) if v inexact."""
    Vr, Ur = [], []
    for v, u, v_exact, u_exact in dims:
        v0, v1 = (v, None) if v_exact else _lev2(v)
        u0, u1 = (u, None) if u_exact else _lev2(u)
        Vr.append(v0); Ur.append(u0)
        if u1 is not None:
            Vr.append(v0); Ur.append(u1)
        if v1 is not None:
            Vr.append(v1); Ur.append(u0)
    return np.stack(Vr), np.stack(Ur)


def _tile_vu(xi, ri, kbi, xj, rj, bj, lj, k16=None):
    """arg = lnf_j - (beta_j + k beta_i) d2, per-tile-centered coords.
    kbi = k_c * beta_i.  Row 0 pairs V=1 with the lnf dim (dead-col hook).
    k16: if set, scale the whole argument by k16 (Schraudolph slots) --
    applied to the U side, except the exact-U dim whose V absorbs it, so
    the expanded row count stays K."""
    one_i = np.ones_like(ri)
    one_j = np.ones_like(rj)
    g = 1.0 if k16 is None else k16
    dims = [
        (one_i, g * (lj - bj * rj), True, False),        # rows 0,1
        (ri, g * -bj, False, False),
        (2.0 * xi[:, 0], g * bj * xj[:, 0], False, False),
        (2.0 * xi[:, 1], g * bj * xj[:, 1], False, False),
        (2.0 * xi[:, 2], g * bj * xj[:, 2], False, False),
        (g * -kbi * ri, one_j, False, True),
        (-kbi, g * rj, False, False),
        (2.0 * kbi * xi[:, 0], g * xj[:, 0], False, False),
        (2.0 * kbi * xi[:, 1], g * xj[:, 1], False, False),
        (2.0 * kbi * xi[:, 2], g * xj[:, 2], False, False),
    ]
    return _expand_rows(dims)


def _prep_inputs(rho, gamma, coords, weights):
    rho = np.asarray(rho, np.float64)
    gamma = np.asarray(gamma, np.float64)
    coords = np.asarray(coords, np.float64)
    weights = np.asarray(weights, np.float64)
    n = rho.shape[0]
    beta, f, lnf, norms = _derived(rho, gamma, weights)

    order = _kd_order(coords)
    cs, bs_, lnfs, fs = coords[order], beta[order], lnf[order], f[order]
    nib = n // IB
    ib_lo = cs.reshape(nib, IB, 3).min(1)
    ib_hi = cs.reshape(nib, IB, 3).max(1)
    ib_c = 0.5 * (ib_lo + ib_hi)

    # exact worst-row culling bound, per (block, channel, j)
    cs32 = np.ascontiguousarray(cs, np.float32)
    bs32 = bs_.astype(np.float32)
    fs32 = fs.astype(np.float32)
    r32 = (cs32 ** 2).sum(1)
    units = []   # (block, channel, sorted alive j indices)
    for b in range(nib):
        ii = slice(b * IB, (b + 1) * IB)
        d2 = np.maximum(r32[ii][:, None] + r32[None, :]
                        - 2.0 * (cs32[ii] @ cs32.T), 0.0)
        for c, kc in enumerate(KCS):
            E = (bs32[None, :] + np.float32(kc) * bs32[ii][:, None]) * d2
            bound = fs32 * np.exp(-np.minimum(E.min(0), 80.0))
            srt = np.argsort(bound)
            cum = np.cumsum(bound[srt].astype(np.float64))
            nd = int(np.searchsorted(cum, EPS_DROP))
            idx = np.sort(srt[nd:])
            if len(idx):
                units.append((b, c, idx))

    # units -> 256-wide slots, distributed unit-wise across cores (LPT)
    unit_slots = [(b, c, idx, (len(idx) + W_SLOT - 1) // W_SLOT)
                  for b, c, idx in units]
    unit_slots.sort(key=lambda u: -u[3])
    core_units = [[] for _ in range(N_CORES)]
    loads = np.zeros(N_CORES, np.int64)
    for b, c, idx, ns in unit_slots:
        m = int(np.argmin(loads))
        loads[m] += ns
        core_units[m].append((b, c, idx, ns))
    S = int(-(-loads.max() // SPT) * SPT)     # slots per core, 8-aligned
    T = S // SPT
    dve_set = set(_dve_tiles(T))
    schraud = [s for s in range(S) if s // SPT in dve_set]
    is_schraud = np.zeros(S, bool)
    is_schraud[schraud] = True

    in_maps = []
    slot_map = []   # per core: (block, channel) per slot
    for m in range(N_CORES):
        U = np.zeros((K, S * W_SLOT), np.float64)
        V = np.zeros((K, S * IB), np.float64)
        smap = []
        s = 0
        for b, c, idx, ns in core_units[m]:
            c_t = ib_c[b]
            ii = slice(b * IB, (b + 1) * IB)
            xi = cs[ii] - c_t
            ri = (xi ** 2).sum(1)
            kbi = KCS[c] * bs_[ii]
            for sl in range(ns):               # build per slot
                sub = idx[sl * W_SLOT:(sl + 1) * W_SLOT]
                real = np.zeros(W_SLOT, bool)
                real[:len(sub)] = True
                xj = np.zeros((W_SLOT, 3)); rj = np.zeros(W_SLOT)
                bj = np.zeros(W_SLOT); lj = np.full(W_SLOT, LNF_DEAD)
                xj[real] = cs[sub] - c_t
                rj[real] = (xj[real] ** 2).sum(1)
                bj[real] = bs_[sub]
                lj[real] = lnfs[sub]
                k16 = K16 if is_schraud[s] else None
                Vt, Ut = _tile_vu(xi, ri, kbi, xj, rj, bj, lj, k16=k16)
                U[:, s * W_SLOT:(s + 1) * W_SLOT] = Ut
                V[:, s * IB:(s + 1) * IB] = Vt
                smap.append((b, c))
                s += 1
        for sd in range(s, S):                 # dead padding slots
            V[0:2, sd * IB:(sd + 1) * IB] = 1.0
            U[0, sd * W_SLOT:(sd + 1) * W_SLOT] = \
                (K16 * LNF_DEAD) if is_schraud[sd] else LNF_DEAD
            smap.append((-1, -1))
        slot_map.append(smap)
        bft = ml_dtypes.bfloat16
        in_maps.append({
            "u": np.ascontiguousarray(U.astype(bft)),
            "v": np.ascontiguousarray(V.astype(bft)),
        })
    meta = dict(order=order, slot_map=slot_map, norms=norms, n=n, S=S, T=T)
    return meta, in_maps


# ---------------------------------------------------------------------------
# Device kernel
# ---------------------------------------------------------------------------

def _build_nc(S, repeat=1):
    import concourse.bass as bass  # noqa: F401
    import concourse.tile as tile
    from concourse import bacc, mybir

    T = S // SPT
    nc = bacc.Bacc("TRN2", target_bir_lowering=False)
    u_dram = nc.dram_tensor("u", [K, S * W_SLOT], mybir.dt.bfloat16,
                            kind="ExternalInput")
    v_dram = nc.dram_tensor("v", [K, S * IB], mybir.dt.bfloat16,
                            kind="ExternalInput")
    y_dram = nc.dram_tensor("y", [IB, S], mybir.dt.float32,
                            kind="ExternalOutput")

    NDMA = 8
    mult = mybir.AluOpType.mult
    add_op = mybir.AluOpType.add

    with tile.TileContext(nc) as tc:
        with (
            tc.tile_pool(name="singles", bufs=1) as singles,
            tc.tile_pool(name="ps", bufs=2, space="PSUM") as ps_pool,
            tc.tile_pool(name="sg", bufs=8) as sg_pool,
        ):
            warm = singles.tile([128, 1], mybir.dt.float32)
            nc.vector.memset(warm[:], 0.0)
            nc.scalar.activation(out=warm[:], in_=warm[:],
                                 func=mybir.ActivationFunctionType.Exp)

            # geometric chunk staging: tiny first chunks so compute starts
            # immediately, larger later ones to bound DMA count
            bounds = [0]
            sz = 1
            while bounds[-1] < T:
                bounds.append(min(T, bounds[-1] + sz))
                sz *= 2
            chunk_of = np.zeros(T, np.int64)
            for ci in range(len(bounds) - 1):
                chunk_of[bounds[ci]:bounds[ci + 1]] = ci
            u_tiles, v_tiles = [], []
            for ci in range(len(bounds) - 1):
                lo, hi = bounds[ci], bounds[ci + 1]
                vt = singles.tile([K, (hi - lo) * SPT * IB],
                                  mybir.dt.bfloat16, tag=f"v{ci}")
                nc.sync.dma_start(vt[:], v_dram[:, lo * SPT * IB:
                                                hi * SPT * IB])
                v_tiles.append(vt)
                ut = singles.tile([K, (hi - lo) * W_TILE],
                                  mybir.dt.bfloat16, tag=f"u{ci}")
                nc.sync.dma_start(ut[:], u_dram[:, lo * W_TILE:hi * W_TILE])
                u_tiles.append(ut)
            parts = singles.tile([IB, S], mybir.dt.float32)

            Exp = mybir.ActivationFunctionType.Exp
            amax = mybir.AluOpType.max
            dve_set = set(_dve_tiles(T))
            for _ in range(repeat):
                for t in range(T):
                    ci = int(chunk_of[t])
                    uc = u_tiles[ci]
                    vc = v_tiles[ci]
                    ubase = (t - bounds[ci]) * W_TILE
                    vbase = (t - bounds[ci]) * SPT * IB
                    pt = ps_pool.tile([IB, W_TILE], mybir.dt.float32,
                                      tag="ps")
                    for s8 in range(SPT):
                        nc.tensor.matmul(
                            pt[:, s8 * W_SLOT:(s8 + 1) * W_SLOT],
                            vc[:, vbase + s8 * IB:vbase + (s8 + 1) * IB],
                            uc[:, ubase + s8 * W_SLOT:
                               ubase + (s8 + 1) * W_SLOT],
                            start=True, stop=True)
                    if t in dve_set:
                        # Schraudolph exp: (arg*K16 + B16) clamped, converted
                        # to int16, bitcast to bf16 (K16 folded host-side)
                        sgi = sg_pool.tile([IB, W_TILE], mybir.dt.int16,
                                           tag="sg")
                        nc.vector.tensor_scalar(
                            out=sgi[:], in0=pt[:], scalar1=float(B16),
                            scalar2=0.0, op0=add_op, op1=amax)
                        sg = sgi[:].bitcast(mybir.dt.bfloat16)
                    else:
                        sgt = sg_pool.tile([IB, W_TILE], mybir.dt.bfloat16,
                                           tag="sg")
                        nc.scalar.activation(out=sgt[:], in_=pt[:], func=Exp)
                        sg = sgt[:]
                    for s8 in range(SPT):
                        s = t * SPT + s8
                        hv = sg[:, s8 * W_SLOT:(s8 + 1) * W_SLOT]
                        nc.vector.tensor_scalar(
                            out=hv, in0=hv, scalar1=1.0, scalar2=0.0,
                            op0=mult, op1=add_op,
                            accum_out=parts[:, s:s + 1])
                    if t == T - 2:   # overlap most of the output writeback
                        mid = (T - 1) * SPT
                        nc.sync.dma_start(y_dram[:, 0:mid], parts[:, 0:mid])
            mid = (T - 1) * SPT if T >= 2 else 0
            nc.sync.dma_start(y_dram[:, mid:S], parts[:, mid:S])
    nc.finalize()
    return nc


def _assemble(meta, results):
    n = meta["n"]
    order, slot_map, norms = meta["order"], meta["slot_map"], meta["norms"]
    Ys = np.zeros((n, 3), np.float64)
    for m, res in enumerate(results):
        y_dev = np.asarray(res["y"], np.float64)       # [128, S]
        for s, (b, c) in enumerate(slot_map[m]):
            if b < 0:
                continue
            Ys[b * IB:(b + 1) * IB, c] += y_dev[:, s]
    Ys *= norms[None, :]
    out = np.empty((n, 3), np.float32)
    out[order] = Ys.astype(np.float32)
    return out


def kernel_run(rho, gamma, coords, weights, **spmd_kwargs):
    """Run on hardware; returns (y, BassKernelResults)."""
    from concourse.bass_utils import run_bass_kernel_spmd

    meta, in_maps = _prep_inputs(rho, gamma, coords, weights)
    key = meta["S"]
    if key not in _NC_CACHE:
        _NC_CACHE[key] = _build_nc(key)
    res = run_bass_kernel_spmd(_NC_CACHE[key], in_maps,
                               core_ids=list(range(N_CORES)), **spmd_kwargs)
    return _assemble(meta, res.results), res


def kernel(rho, gamma, coords, weights):
    y, _ = kernel_run(rho, gamma, coords, weights)
    return y


# revision 40
# speedup vs baseline: 1.4289x; 1.0045x over previous
"""Trainium2 Bass kernel for nn_CiderFeatures (all-pairs Gaussian reduction).

y[i, c] = norms[c] * sum_j exp(-(a_j + b[i,c]) * ||x_i - x_j||^2) * f_j

Key structure (from the reference constants A=D=2):
  a_j = beta_j  and  b_i = (beta_i/2, beta_i, 2*beta_i)  with
  beta = pi*(rho/2)^(2/3) * (2 + C2 * x),  so each channel weight is
  W_c[i,j] = exp(lnf_j - (beta_j + k_c beta_i) d2),  k_c in {1/2, 1, 2}.

Algorithm (identical program on all 8 cores, per-core data):
  - Host: balanced KD-tree sort -> 128-row i-blocks with tight boxes.
    Per (block, channel, j) culling with the EXACT worst-row bound
    f_j * exp(-min_i (beta_j + k_c beta_i) d2_ij), dropping the smallest
    until the dropped mass reaches EPS_DROP per row -- the Gaussians die
    within ~2 units while the cloud has radius ~9, so only ~4.4% of
    (pair, channel) terms survive.
  - Each (block, channel) unit's alive j's are packed into 256-wide
    SLOTS (last slot padded with dead columns); slots are distributed
    across cores unit-wise (LPT) and concatenated into a uniform stream
    of 2048-wide tiles (8 slots each).  All cores run the identical
    instruction stream on per-core packed data; slot counts are padded.
  - Device, per tile: 8 bf16 matmuls (one per slot, K=28 contraction
    rows: 10 logical dims x 2-level bf16 splits, per-tile-centered
    coords, channel scale folded into the V side as exact powers of two)
    fill a [128, 2048] PSUM tile with exp arguments; ONE ScalarE exp
    covers the whole tile (fixed costs amortized 8x); VectorE runs one
    4x-mode tensor_scalar per slot whose free accumulator emits the
    [128,1] j-sum.  ScalarE is the bottleneck at ~95% occupancy.
  - Host scatters the per-slot partials to rows, applies norms, undoes
    the sort.
"""

import numpy as np
import ml_dtypes
from math import pi

N_CORES = 8
IB = 128              # i-block rows (partition dim)
W_SLOT = 256          # unit slot width
SPT = 8               # slots per 2048-wide tile
W_TILE = W_SLOT * SPT
K = 28                # contraction rows (10 dims, 2-level bf16 splits)
EPS_DROP = 5e-3       # max dropped |mass| per row per channel (absolute)
LNF_DEAD = -100.0
KCS = (0.5, 1.0, 2.0)   # channel scales k_c

# int16-Schraudolph exp on VectorE for a fraction of tiles: bf16 bits of
# exp(arg) ~= int16(arg * 128/ln2 + B16), computed as a DVE convert with
# the bias+clamp fused; K16 is folded into the matmul host-side.
K16 = 128.0 / np.log(2.0)
B16 = 128.0 * (127.0 - 0.0583)   # sigma tuned for ~zero mass-weighted bias


def _dve_tiles(T):
    """Tile indices whose exp runs on VectorE (engine balance)."""
    Y = int(round(T / 4.0))
    return sorted({min(T - 1, int((i + 0.5) * T / Y)) for i in range(Y)}) \
        if Y else []


_NC_CACHE = {}


# ---------------------------------------------------------------------------
# Host-side math
# ---------------------------------------------------------------------------

def _derived(rho, gamma, weights):
    B2 = 2.0
    C2 = (6.0 * pi ** 2) ** (2.0 / 3.0) * (6.0 * 2.0 / (160.0 * pi))
    rho_ = rho + 1e-8
    x = (gamma / (8.0 * rho_)) / (0.3 * (3.0 * pi ** 2) ** (2.0 / 3.0)
                                  * rho_ ** (5.0 / 3.0))
    scale = pi * (rho_ / 2.0) ** (2.0 / 3.0)
    beta = scale * (B2 + C2 * x)
    f = weights * rho
    lnf = np.maximum(np.log(np.maximum(f, 1e-300)), LNF_DEAD)
    Bs = np.array([2.0, 1.0, 2.0, 4.0])
    norms = ((Bs[0] + Bs[1:]) / 2.0) ** 1.5
    return beta, f, lnf, norms


def _kd_order(c, leaf=IB):
    """Balanced KD-tree order: leaves of `leaf` points with tight boxes."""
    out = []

    def rec(ids):
        if len(ids) <= leaf:
            out.append(ids)
            return
        ext = c[ids].max(0) - c[ids].min(0)
        srt = ids[np.argsort(c[ids, int(np.argmax(ext))], kind="stable")]
        half = (len(ids) // 2) // leaf * leaf
        if half == 0:
            half = leaf
        rec(srt[:half])
        rec(srt[half:])

    rec(np.arange(len(c)))
    return np.concatenate(out)


def _lev2(M):
    h0 = np.asarray(M, ml_dtypes.bfloat16).astype(np.float64)
    h1 = np.asarray(M - h0, ml_dtypes.bfloat16).astype(np.float64)
    return h0, h1


def _expand_rows(dims):
    """Rows: (v0,u0) always, (v0,u1) if u inexact, (v1,u0) if v inexact."""
    Vr, Ur = [], []
    for v, u, v_exact, u_exact in dims:
        v0, v1 = (v, None) if v_exact else _lev2(v)
        u0, u1 = (u, None) if u_exact else _lev2(u)
        Vr.append(v0); Ur.append(u0)
        if u1 is not None:
            Vr.append(v0); Ur.append(u1)
        if v1 is not None:
            Vr.append(v1); Ur.append(u0)
    return np.stack(Vr), np.stack(Ur)


def _tile_vu(xi, ri, kbi, xj, rj, bj, lj, k16=None):
    """arg = lnf_j - (beta_j + k beta_i) d2, per-tile-centered coords.
    kbi = k_c * beta_i.  Row 0 pairs V=1 with the lnf dim (dead-col hook).
    k16: if set, scale the whole argument by k16 (Schraudolph slots) --
    applied to the U side, except the exact-U dim whose V absorbs it, so
    the expanded row count stays K."""
    one_i = np.ones_like(ri)
    one_j = np.ones_like(rj)
    g = 1.0 if k16 is None else k16
    dims = [
        (one_i, g * (lj - bj * rj), True, False),        # rows 0,1
        (ri, g * -bj, False, False),
        (2.0 * xi[:, 0], g * bj * xj[:, 0], False, False),
        (2.0 * xi[:, 1], g * bj * xj[:, 1], False, False),
        (2.0 * xi[:, 2], g * bj * xj[:, 2], False, False),
        (g * -kbi * ri, one_j, False, True),
        (-kbi, g * rj, False, False),
        (2.0 * kbi * xi[:, 0], g * xj[:, 0], False, False),
        (2.0 * kbi * xi[:, 1], g * xj[:, 1], False, False),
        (2.0 * kbi * xi[:, 2], g * xj[:, 2], False, False),
    ]
    return _expand_rows(dims)


def _prep_inputs(rho, gamma, coords, weights):
    rho = np.asarray(rho, np.float64)
    gamma = np.asarray(gamma, np.float64)
    coords = np.asarray(coords, np.float64)
    weights = np.asarray(weights, np.float64)
    n = rho.shape[0]
    beta, f, lnf, norms = _derived(rho, gamma, weights)

    order = _kd_order(coords)
    cs, bs_, lnfs, fs = coords[order], beta[order], lnf[order], f[order]
    nib = n // IB
    ib_lo = cs.reshape(nib, IB, 3).min(1)
    ib_hi = cs.reshape(nib, IB, 3).max(1)
    ib_c = 0.5 * (ib_lo + ib_hi)

    # exact worst-row culling bound, per (block, channel, j)
    cs32 = np.ascontiguousarray(cs, np.float32)
    bs32 = bs_.astype(np.float32)
    fs32 = fs.astype(np.float32)
    r32 = (cs32 ** 2).sum(1)
    units = []   # (block, channel, sorted alive j indices)
    for b in range(nib):
        ii = slice(b * IB, (b + 1) * IB)
        d2 = np.maximum(r32[ii][:, None] + r32[None, :]
                        - 2.0 * (cs32[ii] @ cs32.T), 0.0)
        for c, kc in enumerate(KCS):
            E = (bs32[None, :] + np.float32(kc) * bs32[ii][:, None]) * d2
            bound = fs32 * np.exp(-np.minimum(E.min(0), 80.0))
            srt = np.argsort(bound)
            cum = np.cumsum(bound[srt].astype(np.float64))
            nd = int(np.searchsorted(cum, EPS_DROP))
            idx = np.sort(srt[nd:])
            if len(idx):
                units.append((b, c, idx))

    # units -> 256-wide slots, distributed unit-wise across cores (LPT)
    unit_slots = [(b, c, idx, (len(idx) + W_SLOT - 1) // W_SLOT)
                  for b, c, idx in units]
    unit_slots.sort(key=lambda u: -u[3])
    core_units = [[] for _ in range(N_CORES)]
    loads = np.zeros(N_CORES, np.int64)
    for b, c, idx, ns in unit_slots:
        m = int(np.argmin(loads))
        loads[m] += ns
        core_units[m].append((b, c, idx, ns))
    S = int(-(-loads.max() // SPT) * SPT)     # slots per core, 8-aligned
    T = S // SPT
    dve_set = set(_dve_tiles(T))
    schraud = [s for s in range(S) if s // SPT in dve_set]
    is_schraud = np.zeros(S, bool)
    is_schraud[schraud] = True

    in_maps = []
    slot_map = []   # per core: (block, channel) per slot
    for m in range(N_CORES):
        U = np.zeros((K, S * W_SLOT), np.float64)
        V = np.zeros((K, S * IB), np.float64)
        smap = []
        s = 0
        for b, c, idx, ns in core_units[m]:
            c_t = ib_c[b]
            ii = slice(b * IB, (b + 1) * IB)
            xi = cs[ii] - c_t
            ri = (xi ** 2).sum(1)
            kbi = KCS[c] * bs_[ii]
            for sl in range(ns):               # build per slot
                sub = idx[sl * W_SLOT:(sl + 1) * W_SLOT]
                real = np.zeros(W_SLOT, bool)
                real[:len(sub)] = True
                xj = np.zeros((W_SLOT, 3)); rj = np.zeros(W_SLOT)
                bj = np.zeros(W_SLOT); lj = np.full(W_SLOT, LNF_DEAD)
                xj[real] = cs[sub] - c_t
                rj[real] = (xj[real] ** 2).sum(1)
                bj[real] = bs_[sub]
                lj[real] = lnfs[sub]
                k16 = K16 if is_schraud[s] else None
                Vt, Ut = _tile_vu(xi, ri, kbi, xj, rj, bj, lj, k16=k16)
                U[:, s * W_SLOT:(s + 1) * W_SLOT] = Ut
                V[:, s * IB:(s + 1) * IB] = Vt
                smap.append((b, c))
                s += 1
        for sd in range(s, S):                 # dead padding slots
            V[0:2, sd * IB:(sd + 1) * IB] = 1.0
            U[0, sd * W_SLOT:(sd + 1) * W_SLOT] = \
                (K16 * LNF_DEAD) if is_schraud[sd] else LNF_DEAD
            smap.append((-1, -1))
        slot_map.append(smap)
        bft = ml_dtypes.bfloat16
        in_maps.append({
            "u": np.ascontiguousarray(U.astype(bft)),
            "v": np.ascontiguousarray(V.astype(bft)),
        })
    meta = dict(order=order, slot_map=slot_map, norms=norms, n=n, S=S, T=T)
    return meta, in_maps


# ---------------------------------------------------------------------------
# Device kernel
# ---------------------------------------------------------------------------

def _build_nc(S, repeat=1):
    import concourse.bass as bass  # noqa: F401
    import concourse.tile as tile
    from concourse import bacc, mybir

    T = S // SPT
    nc = bacc.Bacc("TRN2", target_bir_lowering=False)
    u_dram = nc.dram_tensor("u", [K, S * W_SLOT], mybir.dt.bfloat16,
                            kind="ExternalInput")
    v_dram = nc.dram_tensor("v", [K, S * IB], mybir.dt.bfloat16,
                            kind="ExternalInput")
    y_dram = nc.dram_tensor("y", [IB, S], mybir.dt.float32,
                            kind="ExternalOutput")

    NDMA = 8
    mult = mybir.AluOpType.mult
    add_op = mybir.AluOpType.add

    with tile.TileContext(nc) as tc:
        with (
            tc.tile_pool(name="singles", bufs=1) as singles,
            tc.tile_pool(name="ps", bufs=2, space="PSUM") as ps_pool,
            tc.tile_pool(name="sg", bufs=8) as sg_pool,
        ):
            warm = singles.tile([128, 1], mybir.dt.float32)
            nc.vector.memset(warm[:], 0.0)
            nc.scalar.activation(out=warm[:], in_=warm[:],
                                 func=mybir.ActivationFunctionType.Exp)

            # geometric chunk staging: tiny first chunks so compute starts
            # immediately, larger later ones to bound DMA count
            bounds = [0]
            for szz in (1, 1, 1, 1, 4, 4, 8):
                if bounds[-1] >= T:
                    break
                bounds.append(min(T, bounds[-1] + szz))
            while bounds[-1] < T:
                bounds.append(min(T, bounds[-1] + 8))
            chunk_of = np.zeros(T, np.int64)
            for ci in range(len(bounds) - 1):
                chunk_of[bounds[ci]:bounds[ci + 1]] = ci
            u_tiles, v_tiles = [], []
            for ci in range(len(bounds) - 1):
                lo, hi = bounds[ci], bounds[ci + 1]
                vt = singles.tile([K, (hi - lo) * SPT * IB],
                                  mybir.dt.bfloat16, tag=f"v{ci}")
                nc.sync.dma_start(vt[:], v_dram[:, lo * SPT * IB:
                                                hi * SPT * IB])
                v_tiles.append(vt)
                ut = singles.tile([K, (hi - lo) * W_TILE],
                                  mybir.dt.bfloat16, tag=f"u{ci}")
                nc.sync.dma_start(ut[:], u_dram[:, lo * W_TILE:hi * W_TILE])
                u_tiles.append(ut)
            parts = singles.tile([IB, S], mybir.dt.float32)

            Exp = mybir.ActivationFunctionType.Exp
            amax = mybir.AluOpType.max
            dve_set = set(_dve_tiles(T))
            for _ in range(repeat):
                for t in range(T):
                    ci = int(chunk_of[t])
                    uc = u_tiles[ci]
                    vc = v_tiles[ci]
                    ubase = (t - bounds[ci]) * W_TILE
                    vbase = (t - bounds[ci]) * SPT * IB
                    pt = ps_pool.tile([IB, W_TILE], mybir.dt.float32,
                                      tag="ps")
                    for s8 in range(SPT):
                        nc.tensor.matmul(
                            pt[:, s8 * W_SLOT:(s8 + 1) * W_SLOT],
                            vc[:, vbase + s8 * IB:vbase + (s8 + 1) * IB],
                            uc[:, ubase + s8 * W_SLOT:
                               ubase + (s8 + 1) * W_SLOT],
                            start=True, stop=True)
                    if t in dve_set:
                        # Schraudolph exp: (arg*K16 + B16) clamped, converted
                        # to int16, bitcast to bf16 (K16 folded host-side)
                        sgi = sg_pool.tile([IB, W_TILE], mybir.dt.int16,
                                           tag="sg")
                        nc.vector.tensor_scalar(
                            out=sgi[:], in0=pt[:], scalar1=float(B16),
                            scalar2=0.0, op0=add_op, op1=amax)
                        sg = sgi[:].bitcast(mybir.dt.bfloat16)
                    else:
                        sgt = sg_pool.tile([IB, W_TILE], mybir.dt.bfloat16,
                                           tag="sg")
                        nc.scalar.activation(out=sgt[:], in_=pt[:], func=Exp)
                        sg = sgt[:]
                    for s8 in range(SPT):
                        s = t * SPT + s8
                        hv = sg[:, s8 * W_SLOT:(s8 + 1) * W_SLOT]
                        nc.vector.tensor_scalar(
                            out=hv, in0=hv, scalar1=1.0, scalar2=0.0,
                            op0=mult, op1=add_op,
                            accum_out=parts[:, s:s + 1])
                    if t == T - 2:   # overlap most of the output writeback
                        mid = (T - 1) * SPT
                        nc.sync.dma_start(y_dram[:, 0:mid], parts[:, 0:mid])
            mid = (T - 1) * SPT if T >= 2 else 0
            nc.sync.dma_start(y_dram[:, mid:S], parts[:, mid:S])
    nc.finalize()
    return nc


def _assemble(meta, results):
    n = meta["n"]
    order, slot_map, norms = meta["order"], meta["slot_map"], meta["norms"]
    Ys = np.zeros((n, 3), np.float64)
    for m, res in enumerate(results):
        y_dev = np.asarray(res["y"], np.float64)       # [128, S]
        for s, (b, c) in enumerate(slot_map[m]):
            if b < 0:
                continue
            Ys[b * IB:(b + 1) * IB, c] += y_dev[:, s]
    Ys *= norms[None, :]
    out = np.empty((n, 3), np.float32)
    out[order] = Ys.astype(np.float32)
    return out


def kernel_run(rho, gamma, coords, weights, **spmd_kwargs):
    """Run on hardware; returns (y, BassKernelResults)."""
    from concourse.bass_utils import run_bass_kernel_spmd

    meta, in_maps = _prep_inputs(rho, gamma, coords, weights)
    key = meta["S"]
    if key not in _NC_CACHE:
        _NC_CACHE[key] = _build_nc(key)
    res = run_bass_kernel_spmd(_NC_CACHE[key], in_maps,
                               core_ids=list(range(N_CORES)), **spmd_kwargs)
    return _assemble(meta, res.results), res


def kernel(rho, gamma, coords, weights):
    y, _ = kernel_run(rho, gamma, coords, weights)
    return y
